# revision 1
# baseline (speedup 1.0000x reference)
"""Trainium2 Bass kernel for nn_DecoupledAttentionWeight.

Computes the five projections q_sem/k_sem/q_geo/k_geo/v of x, applies RoPE to
the geo paths, the per-head sigmoid gate + per-path scaling to q (folded into
the projection weights host-side), and returns (q_cat, k_cat, vh) shaped
(B, H, T, 128) each.

Sharding over 8 NeuronCores: 2-way data-parallel over batch (batches {0,1} /
{2,3}) x 4-way tensor-parallel over heads (4 heads per core). Each core runs
one big [8192 x 2048] @ [2048 x 1536] matmul in fp32r (20-bit float: full PE
speed, ~1e-4 rel err) with the per-head output columns packed as
[q_sem|q_geo|k_sem|k_geo|v] so the sem||geo concat is free, then RoPE on the
geo strips via DVE with broadcast access patterns.
"""
import math
import os
import sys

import numpy as np

for _p in ("/opt/trn_rl_repo", os.path.expanduser("~/.axon_site/_ro/trn_rl_repo")):
    if os.path.isdir(_p) and _p not in sys.path:
        sys.path.insert(0, _p)

import concourse.bacc as bacc
import concourse.mybir as mybir
import concourse.tile as tile
from concourse.bass_utils import run_bass_kernel_spmd

# Problem config (hardcoded from the nn.Module init)
D_MODEL = 2048
N_HEADS = 16
SEM_HD = 64
GEO_HD = 64
HEAD_DIM = 128
ROPE_DIM = 64
ROPE_HALF = ROPE_DIM // 2  # 32
ROPE_BASE = 10000.0
B, T = 4, 4096

# Sharding: 2 row groups (2 batches each) x 4 head groups (4 heads each)
N_CORES = 8
RG, HG = 2, 4
ROWS_PER_CORE = (B * T) // RG          # 8192
HEADS_PER_CORE = N_HEADS // HG         # 4
BLK = SEM_HD + GEO_HD + SEM_HD + GEO_HD + HEAD_DIM  # 384 cols per head
N_CORE = HEADS_PER_CORE * BLK          # 1536
K_TILES = D_MODEL // 128               # 16
M_TILES = ROWS_PER_CORE // 128         # 64
SLAB_MT = 2                            # m_tiles per input DMA slab
SLAB_ROWS = SLAB_MT * 128              # 256
N_SLABS = M_TILES // SLAB_MT           # 32
CHUNK = 512                            # psum bank / matmul moving size
N_CHUNKS = N_CORE // CHUNK             # 3
COS_SLOTS = T // 128                   # 32 distinct cos/sin row-tiles

_f32 = mybir.dt.float32
_f32r = mybir.dt.float32r


def _round_fp32r(a: np.ndarray) -> np.ndarray:
    """Round-to-nearest-even fp32 -> fp32r (11-bit mantissa in top 20 bits)."""
    u = np.ascontiguousarray(a, dtype=np.float32).view(np.uint32)
    lsb = (u >> 12) & np.uint32(1)
    r = (u + np.uint32(0x7FF) + lsb) & np.uint32(0xFFFFF000)
    return r.view(np.float32)


def _build_nc():
    nc = bacc.Bacc("TRN2", target_bir_lowering=False, debug=False, num_devices=1)
    xt_d = nc.dram_tensor("xt", [D_MODEL, ROWS_PER_CORE], _f32r, kind="ExternalInput")
    w_d = nc.dram_tensor("w", [D_MODEL, N_CORE], _f32r, kind="ExternalInput")
    cos_d = nc.dram_tensor("cos", [T, ROPE_HALF], _f32, kind="ExternalInput")
    sin_d = nc.dram_tensor("sin", [T, ROPE_HALF], _f32, kind="ExternalInput")
    q_d = nc.dram_tensor(
        "q", [HEADS_PER_CORE, ROWS_PER_CORE, HEAD_DIM], _f32, kind="ExternalOutput"
    )
    k_d = nc.dram_tensor(
        "k", [HEADS_PER_CORE, ROWS_PER_CORE, HEAD_DIM], _f32, kind="ExternalOutput"
    )
    v_d = nc.dram_tensor(
        "v", [HEADS_PER_CORE, ROWS_PER_CORE, HEAD_DIM], _f32, kind="ExternalOutput"
    )

    with tile.TileContext(nc) as tc:
        with (
            tc.tile_pool(name="wp", bufs=1) as wp,
            tc.tile_pool(name="xp", bufs=3) as xp,
            tc.tile_pool(name="trig", bufs=1) as trigp,
            tc.tile_pool(name="stg", bufs=3) as stgp,
            tc.tile_pool(name="tmp", bufs=2) as tmpp,
            tc.tile_pool(name="ps", bufs=2, space="PSUM") as ps,
        ):
            xt_kd = xt_d.ap().rearrange("(k p) m -> p k m", p=128)
            slab_tiles = {}

            def load_slab(s):
                if s not in slab_tiles:
                    t = xp.tile([128, K_TILES * SLAB_ROWS], _f32r, tag="xt")
                    # scalar HWDGE ring: parallel to the W/output DMAs on sync
                    nc.scalar.dma_start(
                        t[:].rearrange("p (k m) -> p k m", k=K_TILES),
                        xt_kd[:, :, s * SLAB_ROWS:(s + 1) * SLAB_ROWS],
                    )
                    slab_tiles[s] = t
                return slab_tiles[s]

            # First x slab must land before any MM can run: issue it ahead
            # of the W loads on the scalar ring.
            load_slab(0)

            # Weights resident, one tile per k so the first m_tile's k-loop
            # can start as soon as W[k=0] lands instead of stalling on the
            # whole 12.6MB load.
            w_kd = w_d.ap().rearrange("(k p) n -> k p n", p=128)
            w_tiles = []
            for k in range(K_TILES):
                wt = wp.tile([128, N_CORE], _f32r, tag=f"w{k}")
                nc.sync.dma_start(wt[:], w_kd[k])
                w_tiles.append(wt)

            # cos/sin tables resident: [128, slot(32) x 32]
            cos_sb = trigp.tile([128, COS_SLOTS * ROPE_HALF], _f32, tag="cos")
            nc.sync.dma_start(
                cos_sb[:].rearrange("p (s c) -> p s c", s=COS_SLOTS),
                cos_d.ap().rearrange("(s p) c -> p s c", p=128),
            )
            sin_sb = trigp.tile([128, COS_SLOTS * ROPE_HALF], _f32, tag="sin")
            nc.sync.dma_start(
                sin_sb[:].rearrange("p (s c) -> p s c", s=COS_SLOTS),
                sin_d.ap().rearrange("(s p) c -> p s c", p=128),
            )
            cos_v = cos_sb[:].rearrange("p (s c) -> p s c", s=COS_SLOTS)
            sin_v = sin_sb[:].rearrange("p (s c) -> p s c", s=COS_SLOTS)

            for s in range(N_SLABS):
                xt_sb = load_slab(s)
                xt_v = xt_sb[:].rearrange("p (k m) -> p k m", k=K_TILES)

                for i in range(SLAB_MT):
                    mt = s * SLAB_MT + i
                    psum = ps.tile([128, N_CORE], _f32, name="psum", tag="psum")
                    # chunk-outer / k-inner: 16 consecutive accumulating MMs
                    # into the same PSUM bank (no per-MM bank switching)
                    for c in range(N_CHUNKS):
                        for k in range(K_TILES):
                            nc.tensor.matmul(
                                psum[:, c * CHUNK:(c + 1) * CHUNK],
                                xt_v[:, k, i * 128:(i + 1) * 128],
                                w_tiles[k][:, c * CHUNK:(c + 1) * CHUNK],
                                start=(k == 0),
                                stop=(k == K_TILES - 1),
                            )

                    # Postprocess: RoPE on geo strips, copy the rest.
                    # Per-head col layout: [qsem 64|qgeo 64|ksem 64|kgeo 64|v 128]
                    # viewed as (h, t3, c): t3=0 -> q(128), 1 -> k(128), 2 -> v(128)
                    pv = psum[:, :].rearrange(
                        "p (h t c) -> p h t c", h=HEADS_PER_CORE, t=3
                    )
                    stg = stgp.tile([128, N_CORE], _f32, tag="stg")
                    sv = stg[:].rearrange(
                        "p (h t c) -> p h t c", h=HEADS_PER_CORE, t=3
                    )
                    slot = mt % COS_SLOTS
                    cos_bc = (
                        cos_v[:, slot, :]
                        .unsqueeze(1)
                        .unsqueeze(1)
                        .broadcast_to([128, HEADS_PER_CORE, 2, ROPE_HALF])
                    )
                    sin_bc = (
                        sin_v[:, slot, :]
                        .unsqueeze(1)
                        .unsqueeze(1)
                        .broadcast_to([128, HEADS_PER_CORE, 2, ROPE_HALF])
                    )
                    x1 = pv[:, :, 0:2, 64:96]
                    x2 = pv[:, :, 0:2, 96:128]
                    shp = [128, HEADS_PER_CORE, 2, ROPE_HALF]
                    t1 = tmpp.tile(shp, _f32, tag="t1")
                    t2 = tmpp.tile(shp, _f32, tag="t2")
                    t3 = tmpp.tile(shp, _f32, tag="t3")
                    t4 = tmpp.tile(shp, _f32, tag="t4")
                    nc.vector.tensor_mul(t1[:], x1, cos_bc)
                    nc.vector.tensor_mul(t2[:], x2, sin_bc)
                    nc.vector.tensor_mul(t3[:], x2, cos_bc)
                    nc.vector.tensor_mul(t4[:], x1, sin_bc)
                    nc.vector.tensor_sub(sv[:, :, 0:2, 64:96], t1[:], t2[:])
                    nc.vector.tensor_add(sv[:, :, 0:2, 96:128], t3[:], t4[:])
                    # sem halves of q and k
                    nc.any.tensor_copy(sv[:, :, 0:2, 0:64], pv[:, :, 0:2, 0:64])
                    # v
                    nc.any.tensor_copy(sv[:, :, 2, :], pv[:, :, 2, :])

                    m0 = mt * 128
                    for t3_idx, out_d in ((0, q_d), (1, k_d), (2, v_d)):
                        nc.sync.dma_start(
                            out_d.ap()[:, m0:m0 + 128, :].transpose([1, 0, 2]),
                            sv[:, :, t3_idx, :],
                        )

    nc.compile()
    return nc


_NC_CACHE = None
LAST_RESULTS = None


def _get_nc():
    global _NC_CACHE
    if _NC_CACHE is None:
        _NC_CACHE = _build_nc()
    return _NC_CACHE


def _host_tables(pos_offset):
    """cos/sin tables computed exactly as the reference does (f32 jax ops)."""
    import jax
    import jax.numpy as jnp

    with jax.default_device(jax.devices("cpu")[0]):
        inv_freq = ROPE_BASE ** (
            -jnp.arange(0, ROPE_HALF, dtype=jnp.float32) * (2.0 / ROPE_DIM)
        )
        pos = jnp.arange(T, dtype=jnp.float32) + jnp.float32(pos_offset)
        ang = pos[:, None] * inv_freq[None, :]
        cos = np.asarray(jnp.cos(ang), dtype=np.float32)
        sin = np.asarray(jnp.sin(ang), dtype=np.float32)
    return np.ascontiguousarray(cos), np.ascontiguousarray(sin)


def _gate(gate_logit):
    import jax
    import jax.numpy as jnp

    with jax.default_device(jax.devices("cpu")[0]):
        g = np.asarray(
            jax.nn.sigmoid(jnp.asarray(gate_logit, dtype=jnp.float32)),
            dtype=np.float32,
        )
    return g


def kernel(x, wq_sem, wk_sem, wq_geo, wk_geo, wv, gate_logit, pos_offset):
    x = np.asarray(x, dtype=np.float32)
    wq_sem = np.asarray(wq_sem, dtype=np.float32)
    wk_sem = np.asarray(wk_sem, dtype=np.float32)
    wq_geo = np.asarray(wq_geo, dtype=np.float32)
    wk_geo = np.asarray(wk_geo, dtype=np.float32)
    wv = np.asarray(wv, dtype=np.float32)
    pos_off = int(np.asarray(pos_offset))

    g = _gate(gate_logit)  # (16,)
    sem_scale = np.float32(1.0 / math.sqrt(float(SEM_HD)))
    geo_scale = np.float32(1.0 / math.sqrt(float(GEO_HD)))
    q_sem_col = (np.float32(2.0) * g * sem_scale).astype(np.float32)   # per head
    q_geo_col = ((np.float32(2.0) - np.float32(2.0) * g) * geo_scale).astype(
        np.float32
    )

    # Per-core weight slabs, cols per head: [qsem|qgeo|ksem|kgeo|v]
    w_cores = []
    for hg in range(HG):
        cols = []
        for hl in range(HEADS_PER_CORE):
            h = hg * HEADS_PER_CORE + hl
            cols.append(wq_sem[:, h * 64:(h + 1) * 64] * q_sem_col[h])
            cols.append(wq_geo[:, h * 64:(h + 1) * 64] * q_geo_col[h])
            cols.append(wk_sem[:, h * 64:(h + 1) * 64])
            cols.append(wk_geo[:, h * 64:(h + 1) * 64])
            cols.append(wv[:, h * 128:(h + 1) * 128])
        w_cores.append(_round_fp32r(np.concatenate(cols, axis=1)))

    # x^T, rounded to fp32r, split into the two row groups
    xr = _round_fp32r(x.reshape(B * T, D_MODEL))
    xt = xr.T  # (2048, 16384) view
    xt_rg = [
        np.ascontiguousarray(xt[:, rg * ROWS_PER_CORE:(rg + 1) * ROWS_PER_CORE])
        for rg in range(RG)
    ]

    cos, sin = _host_tables(pos_off)

    in_maps = []
    for core in range(N_CORES):
        rg, hg = core // HG, core % HG
        in_maps.append(
            {"xt": xt_rg[rg], "w": w_cores[hg], "cos": cos, "sin": sin}
        )

    nc = _get_nc()
    res = run_bass_kernel_spmd(nc, in_maps, list(range(N_CORES)))
    global LAST_RESULTS
    LAST_RESULTS = res

    q_cat = np.empty((B, N_HEADS, T, HEAD_DIM), np.float32)
    k_cat = np.empty((B, N_HEADS, T, HEAD_DIM), np.float32)
    vh = np.empty((B, N_HEADS, T, HEAD_DIM), np.float32)
    for core in range(N_CORES):
        rg, hg = core // HG, core % HG
        r = res.results[core]
        for name, dst in (("q", q_cat), ("k", k_cat), ("v", vh)):
            # (4, 8192, 128) -> (heads, b_local, T, 128)
            a = r[name].reshape(HEADS_PER_CORE, 2, T, HEAD_DIM)
            dst[
                rg * 2:(rg + 1) * 2,
                hg * HEADS_PER_CORE:(hg + 1) * HEADS_PER_CORE,
            ] = a.transpose(1, 0, 2, 3)
    return q_cat, k_cat, vh



# revision 3
# speedup vs baseline: 1.0794x; 1.0794x over previous
"""Trainium2 Bass kernel for nn_DecoupledAttentionWeight.

Computes the five projections q_sem/k_sem/q_geo/k_geo/v of x, applies RoPE to
the geo paths, the per-head sigmoid gate + per-path scaling to q (folded into
the projection weights host-side), and returns (q_cat, k_cat, vh) shaped
(B, H, T, 128) each.

Sharding over 8 NeuronCores: 2-way data-parallel over batch (batches {0,1} /
{2,3}) x 4-way tensor-parallel over heads (4 heads per core). Each core runs
one big [8192 x 2048] @ [2048 x 1536] matmul in bf16 (full PE rate, ~2e-3
rel err) with the per-head output columns packed as
[q_sem|q_geo|k_sem|k_geo|v] so the sem||geo concat is free, then RoPE on the
geo strips via DVE with broadcast access patterns.

The first slab's matmuls are emitted k-outer so the PE starts work as each
weight k-tile's DMA lands instead of stalling for the whole W load.
"""
import math
import os
import sys

import numpy as np

for _p in ("/opt/trn_rl_repo", os.path.expanduser("~/.axon_site/_ro/trn_rl_repo")):
    if os.path.isdir(_p) and _p not in sys.path:
        sys.path.insert(0, _p)

import ml_dtypes

import concourse.bacc as bacc
import concourse.mybir as mybir
import concourse.tile as tile
from concourse.bass_utils import run_bass_kernel_spmd

# Problem config (hardcoded from the nn.Module init)
D_MODEL = 2048
N_HEADS = 16
SEM_HD = 64
GEO_HD = 64
HEAD_DIM = 128
ROPE_DIM = 64
ROPE_HALF = ROPE_DIM // 2  # 32
ROPE_BASE = 10000.0
B, T = 4, 4096

# Sharding: 2 row groups (2 batches each) x 4 head groups (4 heads each)
N_CORES = 8
RG, HG = 2, 4
ROWS_PER_CORE = (B * T) // RG          # 8192
HEADS_PER_CORE = N_HEADS // HG         # 4
BLK = SEM_HD + GEO_HD + SEM_HD + GEO_HD + HEAD_DIM  # 384 cols per head
N_CORE = HEADS_PER_CORE * BLK          # 1536
K_TILES = D_MODEL // 128               # 16
M_TILES = ROWS_PER_CORE // 128         # 64
SLAB_MT = 2                            # m_tiles per input DMA slab
SLAB_ROWS = SLAB_MT * 128              # 256
N_SLABS = M_TILES // SLAB_MT           # 32
CHUNK = 512                            # psum bank / matmul moving size
N_CHUNKS = N_CORE // CHUNK             # 3
COS_SLOTS = T // 128                   # 32 distinct cos/sin row-tiles

_f32 = mybir.dt.float32
_bf16 = mybir.dt.bfloat16
_np_bf16 = ml_dtypes.bfloat16


def _build_nc():
    nc = bacc.Bacc("TRN2", target_bir_lowering=False, debug=False, num_devices=1)
    xt_d = nc.dram_tensor("xt", [D_MODEL, ROWS_PER_CORE], _bf16, kind="ExternalInput")
    w_d = nc.dram_tensor("w", [D_MODEL, N_CORE], _bf16, kind="ExternalInput")
    cos_d = nc.dram_tensor("cos", [T, ROPE_HALF], _f32, kind="ExternalInput")
    sin_d = nc.dram_tensor("sin", [T, ROPE_HALF], _f32, kind="ExternalInput")
    q_d = nc.dram_tensor(
        "q", [ROWS_PER_CORE, HEADS_PER_CORE, HEAD_DIM], _bf16, kind="ExternalOutput"
    )
    k_d = nc.dram_tensor(
        "k", [ROWS_PER_CORE, HEADS_PER_CORE, HEAD_DIM], _bf16, kind="ExternalOutput"
    )
    v_d = nc.dram_tensor(
        "v", [ROWS_PER_CORE, HEADS_PER_CORE, HEAD_DIM], _bf16, kind="ExternalOutput"
    )

    with tile.TileContext(nc) as tc:
        with (
            tc.tile_pool(name="wp", bufs=1) as wp,
            tc.tile_pool(name="xp", bufs=3) as xp,
            tc.tile_pool(name="trig", bufs=1) as trigp,
            tc.tile_pool(name="stg", bufs=3) as stgp,
            tc.tile_pool(name="tmp", bufs=2) as tmpp,
            tc.tile_pool(name="ps", bufs=2, space="PSUM") as ps,
        ):
            xt_kd = xt_d.ap().rearrange("(k p) m -> p k m", p=128)
            slab_tiles = {}

            def load_slab(s):
                if s not in slab_tiles:
                    t = xp.tile([128, K_TILES * SLAB_ROWS], _bf16, tag="xt")
                    # scalar HWDGE ring: parallel to the W/output DMAs on sync
                    nc.scalar.dma_start(
                        t[:].rearrange("p (k m) -> p k m", k=K_TILES),
                        xt_kd[:, :, s * SLAB_ROWS:(s + 1) * SLAB_ROWS],
                    )
                    slab_tiles[s] = t
                return slab_tiles[s]

            # First x slab must land before any MM can run: issue it ahead
            # of the W loads on the scalar ring.
            load_slab(0)

            # Weights resident, one tile per k so the ramp's k-outer matmuls
            # can start as soon as W[k] lands instead of stalling on the
            # whole load.
            w_kd = w_d.ap().rearrange("(k p) n -> k p n", p=128)
            w_tiles = []
            for k in range(K_TILES):
                wt = wp.tile([128, N_CORE], _bf16, tag=f"w{k}")
                nc.sync.dma_start(wt[:], w_kd[k])
                w_tiles.append(wt)

            # cos/sin tables resident: [128, slot(32) x 32]; on the scalar
            # ring right after slab 0 so they land well before the first
            # postprocess (~20us in).
            cos_sb = trigp.tile([128, COS_SLOTS * ROPE_HALF], _f32, tag="cos")
            nc.scalar.dma_start(
                cos_sb[:].rearrange("p (s c) -> p s c", s=COS_SLOTS),
                cos_d.ap().rearrange("(s p) c -> p s c", p=128),
            )
            sin_sb = trigp.tile([128, COS_SLOTS * ROPE_HALF], _f32, tag="sin")
            nc.scalar.dma_start(
                sin_sb[:].rearrange("p (s c) -> p s c", s=COS_SLOTS),
                sin_d.ap().rearrange("(s p) c -> p s c", p=128),
            )
            cos_v = cos_sb[:].rearrange("p (s c) -> p s c", s=COS_SLOTS)
            sin_v = sin_sb[:].rearrange("p (s c) -> p s c", s=COS_SLOTS)

            def postprocess(mt, psum):
                # Postprocess: RoPE on geo strips, copy the rest.
                # Per-head col layout: [qsem 64|qgeo 64|ksem 64|kgeo 64|v 128]
                # viewed as (h, t3, c): t3=0 -> q(128), 1 -> k(128), 2 -> v(128)
                pv = psum[:, :].rearrange(
                    "p (h t c) -> p h t c", h=HEADS_PER_CORE, t=3
                )
                stg = stgp.tile([128, N_CORE], _bf16, tag="stg")
                sv = stg[:].rearrange(
                    "p (h t c) -> p h t c", h=HEADS_PER_CORE, t=3
                )
                slot = mt % COS_SLOTS
                cos_bc = (
                    cos_v[:, slot, :]
                    .unsqueeze(1)
                    .unsqueeze(1)
                    .broadcast_to([128, HEADS_PER_CORE, 2, ROPE_HALF])
                )
                sin_bc = (
                    sin_v[:, slot, :]
                    .unsqueeze(1)
                    .unsqueeze(1)
                    .broadcast_to([128, HEADS_PER_CORE, 2, ROPE_HALF])
                )
                x1 = pv[:, :, 0:2, 64:96]
                x2 = pv[:, :, 0:2, 96:128]
                shp = [128, HEADS_PER_CORE, 2, ROPE_HALF]
                t1 = tmpp.tile(shp, _f32, tag="t1")
                t2 = tmpp.tile(shp, _f32, tag="t2")
                t3 = tmpp.tile(shp, _f32, tag="t3")
                t4 = tmpp.tile(shp, _f32, tag="t4")
                nc.vector.tensor_mul(t1[:], x1, cos_bc)
                nc.vector.tensor_mul(t2[:], x2, sin_bc)
                nc.vector.tensor_mul(t3[:], x2, cos_bc)
                nc.vector.tensor_mul(t4[:], x1, sin_bc)
                nc.vector.tensor_sub(sv[:, :, 0:2, 64:96], t1[:], t2[:])
                nc.vector.tensor_add(sv[:, :, 0:2, 96:128], t3[:], t4[:])
                # sem halves of q and k
                nc.any.tensor_copy(sv[:, :, 0:2, 0:64], pv[:, :, 0:2, 0:64])
                # v
                nc.any.tensor_copy(sv[:, :, 2, :], pv[:, :, 2, :])

                m0 = mt * 128
                for t3_idx, out_d in ((0, q_d), (1, k_d), (2, v_d)):
                    nc.sync.dma_start(
                        out_d.ap()[m0:m0 + 128, :, :],
                        sv[:, :, t3_idx, :],
                    )

            for s in range(N_SLABS):
                xt_sb = load_slab(s)
                xt_v = xt_sb[:].rearrange("p (k m) -> p k m", k=K_TILES)

                if s == 0:
                    # Ramp: k-outer across both m-tiles and all chunks so the
                    # PE has 6 matmuls of work per arriving W k-tile.
                    pts = [
                        ps.tile([128, N_CORE], _f32, name="psum", tag="psum")
                        for _ in range(SLAB_MT)
                    ]
                    for k in range(K_TILES):
                        for i in range(SLAB_MT):
                            for c in range(N_CHUNKS):
                                nc.tensor.matmul(
                                    pts[i][:, c * CHUNK:(c + 1) * CHUNK],
                                    xt_v[:, k, i * 128:(i + 1) * 128],
                                    w_tiles[k][:, c * CHUNK:(c + 1) * CHUNK],
                                    start=(k == 0),
                                    stop=(k == K_TILES - 1),
                                )
                    for i in range(SLAB_MT):
                        postprocess(i, pts[i])
                    continue

                for i in range(SLAB_MT):
                    mt = s * SLAB_MT + i
                    psum = ps.tile([128, N_CORE], _f32, name="psum", tag="psum")
                    # chunk-outer / k-inner: 16 consecutive accumulating MMs
                    # into the same PSUM bank (no per-MM bank switching)
                    for c in range(N_CHUNKS):
                        for k in range(K_TILES):
                            nc.tensor.matmul(
                                psum[:, c * CHUNK:(c + 1) * CHUNK],
                                xt_v[:, k, i * 128:(i + 1) * 128],
                                w_tiles[k][:, c * CHUNK:(c + 1) * CHUNK],
                                start=(k == 0),
                                stop=(k == K_TILES - 1),
                            )
                    postprocess(mt, psum)

    nc.compile()
    return nc


_NC_CACHE = None
LAST_RESULTS = None


def _get_nc():
    global _NC_CACHE
    if _NC_CACHE is None:
        _NC_CACHE = _build_nc()
    return _NC_CACHE


def _host_tables(pos_offset):
    """cos/sin tables computed exactly as the reference does (f32 jax ops)."""
    import jax
    import jax.numpy as jnp

    with jax.default_device(jax.devices("cpu")[0]):
        inv_freq = ROPE_BASE ** (
            -jnp.arange(0, ROPE_HALF, dtype=jnp.float32) * (2.0 / ROPE_DIM)
        )
        pos = jnp.arange(T, dtype=jnp.float32) + jnp.float32(pos_offset)
        ang = pos[:, None] * inv_freq[None, :]
        cos = np.asarray(jnp.cos(ang), dtype=np.float32)
        sin = np.asarray(jnp.sin(ang), dtype=np.float32)
    return np.ascontiguousarray(cos), np.ascontiguousarray(sin)


def _gate(gate_logit):
    import jax

    with jax.default_device(jax.devices("cpu")[0]):
        import jax.numpy as jnp

        g = np.asarray(
            jax.nn.sigmoid(jnp.asarray(gate_logit, dtype=jnp.float32)),
            dtype=np.float32,
        )
    return g


def kernel(x, wq_sem, wk_sem, wq_geo, wk_geo, wv, gate_logit, pos_offset):
    x = np.asarray(x, dtype=np.float32)
    wq_sem = np.asarray(wq_sem, dtype=np.float32)
    wk_sem = np.asarray(wk_sem, dtype=np.float32)
    wq_geo = np.asarray(wq_geo, dtype=np.float32)
    wk_geo = np.asarray(wk_geo, dtype=np.float32)
    wv = np.asarray(wv, dtype=np.float32)
    pos_off = int(np.asarray(pos_offset))

    g = _gate(gate_logit)  # (16,)
    sem_scale = np.float32(1.0 / math.sqrt(float(SEM_HD)))
    geo_scale = np.float32(1.0 / math.sqrt(float(GEO_HD)))
    q_sem_col = (np.float32(2.0) * g * sem_scale).astype(np.float32)   # per head
    q_geo_col = ((np.float32(2.0) - np.float32(2.0) * g) * geo_scale).astype(
        np.float32
    )

    # Per-core weight slabs, cols per head: [qsem|qgeo|ksem|kgeo|v]
    w_cores = []
    for hg in range(HG):
        cols = []
        for hl in range(HEADS_PER_CORE):
            h = hg * HEADS_PER_CORE + hl
            cols.append(wq_sem[:, h * 64:(h + 1) * 64] * q_sem_col[h])
            cols.append(wq_geo[:, h * 64:(h + 1) * 64] * q_geo_col[h])
            cols.append(wk_sem[:, h * 64:(h + 1) * 64])
            cols.append(wk_geo[:, h * 64:(h + 1) * 64])
            cols.append(wv[:, h * 128:(h + 1) * 128])
        w_cores.append(
            np.ascontiguousarray(np.concatenate(cols, axis=1).astype(_np_bf16))
        )

    # x^T in bf16, split into the two row groups
    xr = x.reshape(B * T, D_MODEL).astype(_np_bf16)
    xt = xr.T  # (2048, 16384) view
    xt_rg = [
        np.ascontiguousarray(xt[:, rg * ROWS_PER_CORE:(rg + 1) * ROWS_PER_CORE])
        for rg in range(RG)
    ]

    cos, sin = _host_tables(pos_off)

    in_maps = []
    for core in range(N_CORES):
        rg, hg = core // HG, core % HG
        in_maps.append(
            {"xt": xt_rg[rg], "w": w_cores[hg], "cos": cos, "sin": sin}
        )

    nc = _get_nc()
    res = run_bass_kernel_spmd(nc, in_maps, list(range(N_CORES)))
    global LAST_RESULTS
    LAST_RESULTS = res

    q_cat = np.empty((B, N_HEADS, T, HEAD_DIM), np.float32)
    k_cat = np.empty((B, N_HEADS, T, HEAD_DIM), np.float32)
    vh = np.empty((B, N_HEADS, T, HEAD_DIM), np.float32)
    for core in range(N_CORES):
        rg, hg = core // HG, core % HG
        r = res.results[core]
        for name, dst in (("q", q_cat), ("k", k_cat), ("v", vh)):
            # (8192, 4, 128) bf16 -> (b_local, heads, T, 128) f32
            a = np.asarray(r[name]).astype(np.float32)
            a = a.reshape(2, T, HEADS_PER_CORE, HEAD_DIM).transpose(0, 2, 1, 3)
            dst[
                rg * 2:(rg + 1) * 2,
                hg * HEADS_PER_CORE:(hg + 1) * HEADS_PER_CORE,
            ] = a
    return q_cat, k_cat, vh


# revision 5
# speedup vs baseline: 1.0921x; 1.0118x over previous
"""Trainium2 Bass kernel for nn_DecoupledAttentionWeight.

Computes the five projections q_sem/k_sem/q_geo/k_geo/v of x, applies RoPE to
the geo paths, the per-head sigmoid gate + per-path scaling to q (folded into
the projection weights host-side), and returns (q_cat, k_cat, vh) shaped
(B, H, T, 128) each.

Sharding over 8 NeuronCores: 2-way data-parallel over batch (batches {0,1} /
{2,3}) x 4-way tensor-parallel over heads (4 heads per core). Each core runs
one big [8192 x 2048] @ [2048 x 1536] matmul in bf16 (full PE rate, ~3e-3
rel err) with the per-head output columns packed as
[q_sem|q_geo|k_sem|k_geo|v] so the sem||geo concat is free, then RoPE on the
geo strips via DVE with broadcast access patterns.

DMA layout choices (from trace analysis):
- x is pre-arranged on the host slab-major ([slab, p, k, m]) so each input
  slab is one DMA with 8KB/partition descriptors.
- cos/sin are pre-arranged [p, slot, c] so they load with 4KB descriptors.
- W is split across the two HWDGE rings (even k on sync, odd k on scalar)
  and the first slab's matmuls are emitted k-outer in expected W arrival
  order, so the PE starts as soon as slab0 + w0 land (~10us).
- Staging is t-major ([p, (q|k|v), h, c]) so output DMAs are fully
  contiguous on both sides (1KB descriptors).
"""
import math
import os
import sys

import numpy as np

for _p in ("/opt/trn_rl_repo", os.path.expanduser("~/.axon_site/_ro/trn_rl_repo")):
    if os.path.isdir(_p) and _p not in sys.path:
        sys.path.insert(0, _p)

import ml_dtypes

import concourse.bacc as bacc
import concourse.mybir as mybir
import concourse.tile as tile
from concourse.bass_utils import run_bass_kernel_spmd

# Problem config (hardcoded from the nn.Module init)
D_MODEL = 2048
N_HEADS = 16
SEM_HD = 64
GEO_HD = 64
HEAD_DIM = 128
ROPE_DIM = 64
ROPE_HALF = ROPE_DIM // 2  # 32
ROPE_BASE = 10000.0
B, T = 4, 4096

# Sharding: 2 row groups (2 batches each) x 4 head groups (4 heads each)
N_CORES = 8
RG, HG = 2, 4
ROWS_PER_CORE = (B * T) // RG          # 8192
HEADS_PER_CORE = N_HEADS // HG         # 4
BLK = SEM_HD + GEO_HD + SEM_HD + GEO_HD + HEAD_DIM  # 384 cols per head
N_CORE = HEADS_PER_CORE * BLK          # 1536
K_TILES = D_MODEL // 128               # 16
M_TILES = ROWS_PER_CORE // 128         # 64
SLAB_MT = 2                            # m_tiles per input DMA slab
SLAB_ROWS = SLAB_MT * 128              # 256
N_SLABS = M_TILES // SLAB_MT           # 32
CHUNK = 512                            # psum bank / matmul moving size
N_CHUNKS = N_CORE // CHUNK             # 3
COS_SLOTS = T // 128                   # 32 distinct cos/sin row-tiles

# Ramp matmul k order = expected W-tile arrival order across the two rings
# (evens stream on sync from t~6us; odds on scalar behind slab0).
RAMP_K_ORDER = [0, 2, 1, 4, 3, 6, 5, 8, 7, 10, 9, 12, 11, 14, 13, 15]

_f32 = mybir.dt.float32
_bf16 = mybir.dt.bfloat16
_np_bf16 = ml_dtypes.bfloat16


def _build_nc():
    nc = bacc.Bacc("TRN2", target_bir_lowering=False, debug=False, num_devices=1)
    xt_d = nc.dram_tensor(
        "xt", [N_SLABS * 128, K_TILES * SLAB_ROWS], _bf16, kind="ExternalInput"
    )
    w_d = nc.dram_tensor("w", [D_MODEL, N_CORE], _bf16, kind="ExternalInput")
    cos_d = nc.dram_tensor(
        "cos", [128, COS_SLOTS * ROPE_HALF], _f32, kind="ExternalInput"
    )
    sin_d = nc.dram_tensor(
        "sin", [128, COS_SLOTS * ROPE_HALF], _f32, kind="ExternalInput"
    )
    q_d = nc.dram_tensor(
        "q", [ROWS_PER_CORE, HEADS_PER_CORE, HEAD_DIM], _bf16, kind="ExternalOutput"
    )
    k_d = nc.dram_tensor(
        "k", [ROWS_PER_CORE, HEADS_PER_CORE, HEAD_DIM], _bf16, kind="ExternalOutput"
    )
    v_d = nc.dram_tensor(
        "v", [ROWS_PER_CORE, HEADS_PER_CORE, HEAD_DIM], _bf16, kind="ExternalOutput"
    )

    with tile.TileContext(nc) as tc:
        with (
            tc.tile_pool(name="wp", bufs=1) as wp,
            tc.tile_pool(name="xp", bufs=3) as xp,
            tc.tile_pool(name="trig", bufs=1) as trigp,
            tc.tile_pool(name="stg", bufs=3) as stgp,
            tc.tile_pool(name="tmp", bufs=2) as tmpp,
            tc.tile_pool(name="ps", bufs=2, space="PSUM") as ps,
        ):
            xt_sd = xt_d.ap().rearrange("(s p) f -> s p f", p=128)
            slab_tiles = {}

            def load_slab(s):
                if s not in slab_tiles:
                    t = xp.tile([128, K_TILES * SLAB_ROWS], _bf16, tag="xt")
                    # scalar HWDGE ring; one fully contiguous 8KB/partition DMA
                    nc.scalar.dma_start(t[:], xt_sd[s])
                    slab_tiles[s] = t
                return slab_tiles[s]

            # Scalar ring opens with slab 0 so the ramp can start.
            load_slab(0)

            # W: even k tiles stream on the sync ring (free until outputs),
            # odd k tiles on the scalar ring behind slab0; slabs 1-2 are
            # interleaved so they land before the ramp finishes.
            w_kd = w_d.ap().rearrange("(k p) n -> k p n", p=128)
            w_tiles = [
                wp.tile([128, N_CORE], _bf16, name=f"w{k}", tag=f"w{k}")
                for k in range(K_TILES)
            ]
            cos_sb = trigp.tile([128, COS_SLOTS * ROPE_HALF], _f32, tag="cos")
            sin_sb = trigp.tile([128, COS_SLOTS * ROPE_HALF], _f32, tag="sin")

            # sync ring order: w0, w2, cos, sin, w4, w6, ..., w14
            nc.sync.dma_start(w_tiles[0][:], w_kd[0])
            nc.sync.dma_start(w_tiles[2][:], w_kd[2])
            nc.sync.dma_start(cos_sb[:], cos_d.ap())
            nc.sync.dma_start(sin_sb[:], sin_d.ap())
            for k in range(4, K_TILES, 2):
                nc.sync.dma_start(w_tiles[k][:], w_kd[k])
            # scalar ring order: (slab0,) w1, w3, w5, w7, slab1, w9..w15, slab2
            for k in (1, 3, 5, 7):
                nc.scalar.dma_start(w_tiles[k][:], w_kd[k])
            load_slab(1)
            for k in (9, 11, 13, 15):
                nc.scalar.dma_start(w_tiles[k][:], w_kd[k])
            load_slab(2)

            cos_v = cos_sb[:].rearrange("p (s c) -> p s c", s=COS_SLOTS)
            sin_v = sin_sb[:].rearrange("p (s c) -> p s c", s=COS_SLOTS)

            def postprocess(mt, psum):
                # RoPE on the geo strips, copy the rest, all in t-major order
                # so stores and output DMAs are contiguous.
                # psum col layout per head: [qsem 64|qgeo 64|ksem 64|kgeo 64|v 128]
                # viewed as (t, h, c): t=0 -> q(128), 1 -> k(128), 2 -> v(128)
                pv = psum[:, :].rearrange(
                    "p (h t c) -> p t h c", h=HEADS_PER_CORE, t=3
                )
                stg = stgp.tile(
                    [128, 3, HEADS_PER_CORE, HEAD_DIM], _bf16, tag="stg"
                )
                sv = stg[:]
                slot = mt % COS_SLOTS
                cos_bc = (
                    cos_v[:, slot, :]
                    .unsqueeze(1)
                    .unsqueeze(1)
                    .broadcast_to([128, 2, HEADS_PER_CORE, ROPE_HALF])
                )
                sin_bc = (
                    sin_v[:, slot, :]
                    .unsqueeze(1)
                    .unsqueeze(1)
                    .broadcast_to([128, 2, HEADS_PER_CORE, ROPE_HALF])
                )
                x1 = pv[:, 0:2, :, 64:96]
                x2 = pv[:, 0:2, :, 96:128]
                shp = [128, 2, HEADS_PER_CORE, ROPE_HALF]
                t1 = tmpp.tile(shp, _f32, tag="t1")
                t2 = tmpp.tile(shp, _f32, tag="t2")
                t3 = tmpp.tile(shp, _f32, tag="t3")
                t4 = tmpp.tile(shp, _f32, tag="t4")
                nc.vector.tensor_mul(t1[:], x1, cos_bc)
                nc.vector.tensor_mul(t2[:], x2, sin_bc)
                nc.vector.tensor_mul(t3[:], x2, cos_bc)
                nc.vector.tensor_mul(t4[:], x1, sin_bc)
                nc.vector.tensor_sub(sv[:, 0:2, :, 64:96], t1[:], t2[:])
                nc.vector.tensor_add(sv[:, 0:2, :, 96:128], t3[:], t4[:])
                # sem halves of q and k
                nc.any.tensor_copy(sv[:, 0:2, :, 0:64], pv[:, 0:2, :, 0:64])
                # v
                nc.any.tensor_copy(sv[:, 2, :, :], pv[:, 2, :, :])

                m0 = mt * 128
                for t_idx, out_d in ((0, q_d), (1, k_d), (2, v_d)):
                    nc.sync.dma_start(
                        out_d.ap()[m0:m0 + 128, :, :],
                        sv[:, t_idx, :, :],
                    )

            for s in range(N_SLABS):
                xt_sb = load_slab(s)
                xt_v = xt_sb[:].rearrange("p (k m) -> p k m", k=K_TILES)

                if s == 0:
                    # Ramp: k-outer (in W arrival order) across both m-tiles
                    # and all chunks: 6 matmuls of work per arriving W tile.
                    pts = [
                        ps.tile([128, N_CORE], _f32, name="psum", tag="psum")
                        for _ in range(SLAB_MT)
                    ]
                    for j, k in enumerate(RAMP_K_ORDER):
                        for i in range(SLAB_MT):
                            for c in range(N_CHUNKS):
                                nc.tensor.matmul(
                                    pts[i][:, c * CHUNK:(c + 1) * CHUNK],
                                    xt_v[:, k, i * 128:(i + 1) * 128],
                                    w_tiles[k][:, c * CHUNK:(c + 1) * CHUNK],
                                    start=(j == 0),
                                    stop=(j == K_TILES - 1),
                                )
                    for i in range(SLAB_MT):
                        postprocess(i, pts[i])
                    continue

                for i in range(SLAB_MT):
                    mt = s * SLAB_MT + i
                    psum = ps.tile([128, N_CORE], _f32, name="psum", tag="psum")
                    # chunk-outer / k-inner: 16 consecutive accumulating MMs
                    # into the same PSUM bank (no per-MM bank switching)
                    for c in range(N_CHUNKS):
                        for k in range(K_TILES):
                            nc.tensor.matmul(
                                psum[:, c * CHUNK:(c + 1) * CHUNK],
                                xt_v[:, k, i * 128:(i + 1) * 128],
                                w_tiles[k][:, c * CHUNK:(c + 1) * CHUNK],
                                start=(k == 0),
                                stop=(k == K_TILES - 1),
                            )
                    postprocess(mt, psum)

    nc.compile()
    return nc


_NC_CACHE = None
LAST_RESULTS = None


def _get_nc():
    global _NC_CACHE
    if _NC_CACHE is None:
        _NC_CACHE = _build_nc()
    return _NC_CACHE


def _host_tables(pos_offset):
    """cos/sin tables computed exactly as the reference does (f32 jax ops),
    pre-arranged to [p, slot, c] so the DMA descriptors are 4KB."""
    import jax
    import jax.numpy as jnp

    with jax.default_device(jax.devices("cpu")[0]):
        inv_freq = ROPE_BASE ** (
            -jnp.arange(0, ROPE_HALF, dtype=jnp.float32) * (2.0 / ROPE_DIM)
        )
        pos = jnp.arange(T, dtype=jnp.float32) + jnp.float32(pos_offset)
        ang = pos[:, None] * inv_freq[None, :]
        cos = np.asarray(jnp.cos(ang), dtype=np.float32)
        sin = np.asarray(jnp.sin(ang), dtype=np.float32)

    def _arr(a):
        # (T, c) -> (p, slot*c) with T = slot*128 + p
        return np.ascontiguousarray(
            a.reshape(COS_SLOTS, 128, ROPE_HALF).transpose(1, 0, 2)
        ).reshape(128, COS_SLOTS * ROPE_HALF)

    return _arr(cos), _arr(sin)


def _gate(gate_logit):
    import jax

    with jax.default_device(jax.devices("cpu")[0]):
        import jax.numpy as jnp

        g = np.asarray(
            jax.nn.sigmoid(jnp.asarray(gate_logit, dtype=jnp.float32)),
            dtype=np.float32,
        )
    return g


def kernel(x, wq_sem, wk_sem, wq_geo, wk_geo, wv, gate_logit, pos_offset):
    x = np.asarray(x, dtype=np.float32)
    wq_sem = np.asarray(wq_sem, dtype=np.float32)
    wk_sem = np.asarray(wk_sem, dtype=np.float32)
    wq_geo = np.asarray(wq_geo, dtype=np.float32)
    wk_geo = np.asarray(wk_geo, dtype=np.float32)
    wv = np.asarray(wv, dtype=np.float32)
    pos_off = int(np.asarray(pos_offset))

    g = _gate(gate_logit)  # (16,)
    sem_scale = np.float32(1.0 / math.sqrt(float(SEM_HD)))
    geo_scale = np.float32(1.0 / math.sqrt(float(GEO_HD)))
    q_sem_col = (np.float32(2.0) * g * sem_scale).astype(np.float32)   # per head
    q_geo_col = ((np.float32(2.0) - np.float32(2.0) * g) * geo_scale).astype(
        np.float32
    )

    # Per-core weight slabs, cols per head: [qsem|qgeo|ksem|kgeo|v]
    w_cores = []
    for hg in range(HG):
        cols = []
        for hl in range(HEADS_PER_CORE):
            h = hg * HEADS_PER_CORE + hl
            cols.append(wq_sem[:, h * 64:(h + 1) * 64] * q_sem_col[h])
            cols.append(wq_geo[:, h * 64:(h + 1) * 64] * q_geo_col[h])
            cols.append(wk_sem[:, h * 64:(h + 1) * 64])
            cols.append(wk_geo[:, h * 64:(h + 1) * 64])
            cols.append(wv[:, h * 128:(h + 1) * 128])
        w_cores.append(
            np.ascontiguousarray(np.concatenate(cols, axis=1).astype(_np_bf16))
        )

    # x in bf16, slab-major per core: xt3[s, p, k, ml] = x_rows[s*256+ml, k*128+p]
    xr = x.reshape(B * T, D_MODEL).astype(_np_bf16)
    xt_rg = []
    for rg in range(RG):
        rows = xr[rg * ROWS_PER_CORE:(rg + 1) * ROWS_PER_CORE]
        t = rows.reshape(N_SLABS, SLAB_ROWS, K_TILES, 128)
        xt_rg.append(
            np.ascontiguousarray(t.transpose(0, 3, 2, 1)).reshape(
                N_SLABS * 128, K_TILES * SLAB_ROWS
            )
        )

    cos, sin = _host_tables(pos_off)

    in_maps = []
    for core in range(N_CORES):
        rg, hg = core // HG, core % HG
        in_maps.append(
            {"xt": xt_rg[rg], "w": w_cores[hg], "cos": cos, "sin": sin}
        )

    nc = _get_nc()
    res = run_bass_kernel_spmd(nc, in_maps, list(range(N_CORES)))
    global LAST_RESULTS
    LAST_RESULTS = res

    q_cat = np.empty((B, N_HEADS, T, HEAD_DIM), np.float32)
    k_cat = np.empty((B, N_HEADS, T, HEAD_DIM), np.float32)
    vh = np.empty((B, N_HEADS, T, HEAD_DIM), np.float32)
    for core in range(N_CORES):
        rg, hg = core // HG, core % HG
        r = res.results[core]
        for name, dst in (("q", q_cat), ("k", k_cat), ("v", vh)):
            # (8192, 4, 128) bf16 -> (b_local, heads, T, 128) f32
            a = np.asarray(r[name]).astype(np.float32)
            a = a.reshape(2, T, HEADS_PER_CORE, HEAD_DIM).transpose(0, 2, 1, 3)
            dst[
                rg * 2:(rg + 1) * 2,
                hg * HEADS_PER_CORE:(hg + 1) * HEADS_PER_CORE,
            ] = a
    return q_cat, k_cat, vh


# revision 6
# speedup vs baseline: 1.1120x; 1.0182x over previous
"""Trainium2 Bass kernel for nn_DecoupledAttentionWeight.

Computes the five projections q_sem/k_sem/q_geo/k_geo/v of x, applies RoPE to
the geo paths, the per-head sigmoid gate + per-path scaling to q (folded into
the projection weights host-side), and returns (q_cat, k_cat, vh) shaped
(B, H, T, 128) each.

Sharding over 8 NeuronCores: 2-way data-parallel over batch (batches {0,1} /
{2,3}) x 4-way tensor-parallel over heads (4 heads per core). Each core runs
one big [8192 x 2048] @ [2048 x 1536] matmul in bf16 (full PE rate, ~3e-3
rel err).

Layout/pipelining choices (from trace analysis):
- W columns are grouped per 512-wide PSUM chunk: chunk0 = [qsem x4 | qgeo x4],
  chunk1 = [ksem x4 | kgeo x4], chunk2 = [v x4]. Each chunk is postprocessed
  independently (uniform strided APs) and DMAd out as soon as it stops, so
  PSUM banks recycle at chunk granularity and the kernel tail is only the
  v-copy + v-DMA of the last m-tile.
- PSUM pool = 8 single-bank tiles; the PE is never more than one bank-free
  wait behind the DVE.
- x is pre-arranged on the host slab-major ([slab, p, k, m]) so each input
  slab is one DMA with 8KB/partition descriptors; slab 0 is split in half so
  the first matmul can start ~2us earlier.
- cos/sin are pre-arranged [p, slot, c] (4KB descriptors), after W on sync.
- W is split across the two HWDGE rings (even k on sync, odd k on scalar)
  and the first slab's matmuls are emitted k-outer so the PE starts as soon
  as slab0a + w0 land (~10us) and works while W streams.
- Staging is t-major ([p, (q|k|v), h, c]) so output DMAs are fully
  contiguous on both sides (1KB descriptors).
"""
import math
import os
import sys

import numpy as np

for _p in ("/opt/trn_rl_repo", os.path.expanduser("~/.axon_site/_ro/trn_rl_repo")):
    if os.path.isdir(_p) and _p not in sys.path:
        sys.path.insert(0, _p)

import ml_dtypes

import concourse.bacc as bacc
import concourse.mybir as mybir
import concourse.tile as tile
from concourse.bass_utils import run_bass_kernel_spmd

# Problem config (hardcoded from the nn.Module init)
D_MODEL = 2048
N_HEADS = 16
SEM_HD = 64
GEO_HD = 64
HEAD_DIM = 128
ROPE_DIM = 64
ROPE_HALF = ROPE_DIM // 2  # 32
ROPE_BASE = 10000.0
B, T = 4, 4096

# Sharding: 2 row groups (2 batches each) x 4 head groups (4 heads each)
N_CORES = 8
RG, HG = 2, 4
ROWS_PER_CORE = (B * T) // RG          # 8192
HEADS_PER_CORE = N_HEADS // HG         # 4
N_CORE = HEADS_PER_CORE * 384          # 1536 packed cols per core
K_TILES = D_MODEL // 128               # 16
M_TILES = ROWS_PER_CORE // 128         # 64
SLAB_MT = 2                            # m_tiles per input DMA slab
SLAB_ROWS = SLAB_MT * 128              # 256
N_SLABS = M_TILES // SLAB_MT           # 32
CHUNK = 512                            # psum bank / matmul moving size
N_CHUNKS = N_CORE // CHUNK             # 3
COS_SLOTS = T // 128                   # 32 distinct cos/sin row-tiles

_f32 = mybir.dt.float32
_bf16 = mybir.dt.bfloat16
_np_bf16 = ml_dtypes.bfloat16


def _build_nc():
    nc = bacc.Bacc("TRN2", target_bir_lowering=False, debug=False, num_devices=1)
    xt_d = nc.dram_tensor(
        "xt", [N_SLABS * 128, K_TILES * SLAB_ROWS], _bf16, kind="ExternalInput"
    )
    w_d = nc.dram_tensor("w", [D_MODEL, N_CORE], _bf16, kind="ExternalInput")
    cos_d = nc.dram_tensor(
        "cos", [128, COS_SLOTS * ROPE_HALF], _f32, kind="ExternalInput"
    )
    sin_d = nc.dram_tensor(
        "sin", [128, COS_SLOTS * ROPE_HALF], _f32, kind="ExternalInput"
    )
    q_d = nc.dram_tensor(
        "q", [ROWS_PER_CORE, HEADS_PER_CORE, HEAD_DIM], _bf16, kind="ExternalOutput"
    )
    k_d = nc.dram_tensor(
        "k", [ROWS_PER_CORE, HEADS_PER_CORE, HEAD_DIM], _bf16, kind="ExternalOutput"
    )
    v_d = nc.dram_tensor(
        "v", [ROWS_PER_CORE, HEADS_PER_CORE, HEAD_DIM], _bf16, kind="ExternalOutput"
    )
    out_ds = (q_d, k_d, v_d)

    with tile.TileContext(nc) as tc:
        with (
            tc.tile_pool(name="wp", bufs=1) as wp,
            tc.tile_pool(name="x0p", bufs=1) as x0p,
            tc.tile_pool(name="xp", bufs=3) as xp,
            tc.tile_pool(name="trig", bufs=1) as trigp,
            tc.tile_pool(name="stg", bufs=3) as stgp,
            tc.tile_pool(name="tmp", bufs=2) as tmpp,
            tc.tile_pool(name="ps", bufs=8, space="PSUM") as ps,
        ):
            xt_sd = xt_d.ap().rearrange("(s p) f -> s p f", p=128)
            slab_tiles = {}

            def load_slab(s):
                if s not in slab_tiles:
                    t = xp.tile([128, K_TILES * SLAB_ROWS], _bf16, tag="xt")
                    # scalar HWDGE ring; one fully contiguous 8KB/partition DMA
                    nc.scalar.dma_start(t[:], xt_sd[s])
                    slab_tiles[s] = t
                return slab_tiles[s]

            # Slab 0 in two halves (k 0..7 / 8..15) so the ramp's first
            # matmul only waits on 512KB.
            HALF_F = (K_TILES // 2) * SLAB_ROWS
            xa = x0p.tile([128, HALF_F], _bf16, name="xa", tag="xa")
            xb = x0p.tile([128, HALF_F], _bf16, name="xb", tag="xb")
            nc.scalar.dma_start(xa[:], xt_sd[0][:, 0:HALF_F])

            w_kd = w_d.ap().rearrange("(k p) n -> k p n", p=128)
            w_tiles = [
                wp.tile([128, N_CORE], _bf16, name=f"w{k}", tag=f"w{k}")
                for k in range(K_TILES)
            ]
            cos_sb = trigp.tile([128, COS_SLOTS * ROPE_HALF], _f32, tag="cos")
            sin_sb = trigp.tile([128, COS_SLOTS * ROPE_HALF], _f32, tag="sin")

            # sync ring: even W tiles, then cos/sin (needed ~30us in), then
            # (during the loop) all output DMAs.
            for k in range(0, K_TILES, 2):
                nc.sync.dma_start(w_tiles[k][:], w_kd[k])
            nc.sync.dma_start(cos_sb[:], cos_d.ap())
            nc.sync.dma_start(sin_sb[:], sin_d.ap())
            # scalar ring: xa, w1, xb, remaining odd W, then slabs 1, 2, ...
            nc.scalar.dma_start(w_tiles[1][:], w_kd[1])
            nc.scalar.dma_start(xb[:], xt_sd[0][:, HALF_F:])
            for k in range(3, K_TILES, 2):
                nc.scalar.dma_start(w_tiles[k][:], w_kd[k])
            load_slab(1)
            load_slab(2)

            cos_v = cos_sb[:].rearrange("p (s c) -> p s c", s=COS_SLOTS)
            sin_v = sin_sb[:].rearrange("p (s c) -> p s c", s=COS_SLOTS)

            def make_stg():
                return stgp.tile(
                    [128, 3, HEADS_PER_CORE, HEAD_DIM], _bf16,
                    name="stg", tag="stg",
                )

            def _sem_geo_post(t_idx, mt, pc, stg):
                # chunk layout: [sem h0..h3 (256) | geo h0..h3 (256)]
                sv = stg[:]
                slot = mt % COS_SLOTS
                nc.any.tensor_copy(
                    sv[:, t_idx, :, 0:64],
                    pc[:, 0:256].rearrange("p (h c) -> p h c", h=HEADS_PER_CORE),
                )
                xg = pc[:, 256:512].rearrange(
                    "p (h l c) -> p h l c", h=HEADS_PER_CORE, l=2
                )
                x1 = xg[:, :, 0, :]
                x2 = xg[:, :, 1, :]
                cos_bc = (
                    cos_v[:, slot, :]
                    .unsqueeze(1)
                    .broadcast_to([128, HEADS_PER_CORE, ROPE_HALF])
                )
                sin_bc = (
                    sin_v[:, slot, :]
                    .unsqueeze(1)
                    .broadcast_to([128, HEADS_PER_CORE, ROPE_HALF])
                )
                shp = [128, HEADS_PER_CORE, ROPE_HALF]
                t1 = tmpp.tile(shp, _f32, name="t1", tag="t1")
                t2 = tmpp.tile(shp, _f32, name="t2", tag="t2")
                t3 = tmpp.tile(shp, _f32, name="t3", tag="t3")
                t4 = tmpp.tile(shp, _f32, name="t4", tag="t4")
                nc.vector.tensor_mul(t1[:], x1, cos_bc)
                nc.vector.tensor_mul(t2[:], x2, sin_bc)
                nc.vector.tensor_mul(t3[:], x2, cos_bc)
                nc.vector.tensor_mul(t4[:], x1, sin_bc)
                nc.vector.tensor_sub(sv[:, t_idx, :, 64:96], t1[:], t2[:])
                nc.vector.tensor_add(sv[:, t_idx, :, 96:128], t3[:], t4[:])
                m0 = mt * 128
                nc.sync.dma_start(
                    out_ds[t_idx].ap()[m0:m0 + 128, :, :], sv[:, t_idx, :, :]
                )

            def post_q(mt, pc, stg):
                _sem_geo_post(0, mt, pc, stg)

            def post_k(mt, pc, stg):
                _sem_geo_post(1, mt, pc, stg)

            def post_v(mt, pc, stg):
                sv = stg[:]
                nc.any.tensor_copy(
                    sv[:, 2, :, :],
                    pc[:, :].rearrange("p (h c) -> p h c", h=HEADS_PER_CORE),
                )
                m0 = mt * 128
                nc.sync.dma_start(v_d.ap()[m0:m0 + 128, :, :], sv[:, 2, :, :])

            POSTS = (post_q, post_k, post_v)

            # ---- Ramp: slab 0, k-outer so the PE works while W streams ----
            rpts = [
                [
                    ps.tile([128, CHUNK], _f32, name=f"ps{i}{c}", tag="psc")
                    for c in range(N_CHUNKS)
                ]
                for i in range(SLAB_MT)
            ]
            for k in range(K_TILES):
                xsb = xa if k < K_TILES // 2 else xb
                xv = xsb[:].rearrange("p (k m) -> p k m", k=K_TILES // 2)
                for i in range(SLAB_MT):
                    for c in range(N_CHUNKS):
                        nc.tensor.matmul(
                            rpts[i][c][:, :],
                            xv[:, k % (K_TILES // 2), i * 128:(i + 1) * 128],
                            w_tiles[k][:, c * CHUNK:(c + 1) * CHUNK],
                            start=(k == 0),
                            stop=(k == K_TILES - 1),
                        )
            for i in range(SLAB_MT):
                stg = make_stg()
                for c in range(N_CHUNKS):
                    POSTS[c](i, rpts[i][c], stg)

            # ---- Steady state ----
            for s in range(1, N_SLABS):
                xt_sb = load_slab(s)
                xt_v = xt_sb[:].rearrange("p (k m) -> p k m", k=K_TILES)
                for i in range(SLAB_MT):
                    mt = s * SLAB_MT + i
                    stg = make_stg()
                    for c in range(N_CHUNKS):
                        pc = ps.tile([128, CHUNK], _f32, name="pc", tag="psc")
                        # k-inner: 16 consecutive accumulating MMs per bank
                        for k in range(K_TILES):
                            nc.tensor.matmul(
                                pc[:, :],
                                xt_v[:, k, i * 128:(i + 1) * 128],
                                w_tiles[k][:, c * CHUNK:(c + 1) * CHUNK],
                                start=(k == 0),
                                stop=(k == K_TILES - 1),
                            )
                        POSTS[c](mt, pc, stg)

    nc.compile()
    return nc


_NC_CACHE = None
LAST_RESULTS = None


def _get_nc():
    global _NC_CACHE
    if _NC_CACHE is None:
        _NC_CACHE = _build_nc()
    return _NC_CACHE


def _host_tables(pos_offset):
    """cos/sin tables computed exactly as the reference does (f32 jax ops),
    pre-arranged to [p, slot, c] so the DMA descriptors are 4KB."""
    import jax
    import jax.numpy as jnp

    with jax.default_device(jax.devices("cpu")[0]):
        inv_freq = ROPE_BASE ** (
            -jnp.arange(0, ROPE_HALF, dtype=jnp.float32) * (2.0 / ROPE_DIM)
        )
        pos = jnp.arange(T, dtype=jnp.float32) + jnp.float32(pos_offset)
        ang = pos[:, None] * inv_freq[None, :]
        cos = np.asarray(jnp.cos(ang), dtype=np.float32)
        sin = np.asarray(jnp.sin(ang), dtype=np.float32)

    def _arr(a):
        # (T, c) -> (p, slot*c) with T = slot*128 + p
        return np.ascontiguousarray(
            a.reshape(COS_SLOTS, 128, ROPE_HALF).transpose(1, 0, 2)
        ).reshape(128, COS_SLOTS * ROPE_HALF)

    return _arr(cos), _arr(sin)


def _gate(gate_logit):
    import jax

    with jax.default_device(jax.devices("cpu")[0]):
        import jax.numpy as jnp

        g = np.asarray(
            jax.nn.sigmoid(jnp.asarray(gate_logit, dtype=jnp.float32)),
            dtype=np.float32,
        )
    return g


def kernel(x, wq_sem, wk_sem, wq_geo, wk_geo, wv, gate_logit, pos_offset):
    x = np.asarray(x, dtype=np.float32)
    wq_sem = np.asarray(wq_sem, dtype=np.float32)
    wk_sem = np.asarray(wk_sem, dtype=np.float32)
    wq_geo = np.asarray(wq_geo, dtype=np.float32)
    wk_geo = np.asarray(wk_geo, dtype=np.float32)
    wv = np.asarray(wv, dtype=np.float32)
    pos_off = int(np.asarray(pos_offset))

    g = _gate(gate_logit)  # (16,)
    sem_scale = np.float32(1.0 / math.sqrt(float(SEM_HD)))
    geo_scale = np.float32(1.0 / math.sqrt(float(GEO_HD)))
    q_sem_col = (np.float32(2.0) * g * sem_scale).astype(np.float32)   # per head
    q_geo_col = ((np.float32(2.0) - np.float32(2.0) * g) * geo_scale).astype(
        np.float32
    )

    # Per-core weight slabs, grouped per 512-chunk:
    # [qsem h0..h3 | qgeo h0..h3] [ksem | kgeo] [v h0..h3]
    w_cores = []
    for hg in range(HG):
        heads = [hg * HEADS_PER_CORE + hl for hl in range(HEADS_PER_CORE)]
        cols = []
        cols += [wq_sem[:, h * 64:(h + 1) * 64] * q_sem_col[h] for h in heads]
        cols += [wq_geo[:, h * 64:(h + 1) * 64] * q_geo_col[h] for h in heads]
        cols += [wk_sem[:, h * 64:(h + 1) * 64] for h in heads]
        cols += [wk_geo[:, h * 64:(h + 1) * 64] for h in heads]
        cols += [wv[:, h * 128:(h + 1) * 128] for h in heads]
        w_cores.append(
            np.ascontiguousarray(np.concatenate(cols, axis=1).astype(_np_bf16))
        )

    # x in bf16, slab-major per core: xt3[s, p, k, ml] = x_rows[s*256+ml, k*128+p]
    xr = x.reshape(B * T, D_MODEL).astype(_np_bf16)
    xt_rg = []
    for rg in range(RG):
        rows = xr[rg * ROWS_PER_CORE:(rg + 1) * ROWS_PER_CORE]
        t = rows.reshape(N_SLABS, SLAB_ROWS, K_TILES, 128)
        xt_rg.append(
            np.ascontiguousarray(t.transpose(0, 3, 2, 1)).reshape(
                N_SLABS * 128, K_TILES * SLAB_ROWS
            )
        )

    cos, sin = _host_tables(pos_off)

    in_maps = []
    for core in range(N_CORES):
        rg, hg = core // HG, core % HG
        in_maps.append(
            {"xt": xt_rg[rg], "w": w_cores[hg], "cos": cos, "sin": sin}
        )

    nc = _get_nc()
    res = run_bass_kernel_spmd(nc, in_maps, list(range(N_CORES)))
    global LAST_RESULTS
    LAST_RESULTS = res

    q_cat = np.empty((B, N_HEADS, T, HEAD_DIM), np.float32)
    k_cat = np.empty((B, N_HEADS, T, HEAD_DIM), np.float32)
    vh = np.empty((B, N_HEADS, T, HEAD_DIM), np.float32)
    for core in range(N_CORES):
        rg, hg = core // HG, core % HG
        r = res.results[core]
        for name, dst in (("q", q_cat), ("k", k_cat), ("v", vh)):
            # (8192, 4, 128) bf16 -> (b_local, heads, T, 128) f32
            a = np.asarray(r[name]).astype(np.float32)
            a = a.reshape(2, T, HEADS_PER_CORE, HEAD_DIM).transpose(0, 2, 1, 3)
            dst[
                rg * 2:(rg + 1) * 2,
                hg * HEADS_PER_CORE:(hg + 1) * HEADS_PER_CORE,
            ] = a
    return q_cat, k_cat, vh


# revision 10
# speedup vs baseline: 1.1125x; 1.0005x over previous
"""Trainium2 Bass kernel for nn_DecoupledAttentionWeight.

Computes the five projections q_sem/k_sem/q_geo/k_geo/v of x, applies RoPE to
the geo paths, the per-head sigmoid gate + per-path scaling to q (folded into
the projection weights host-side), and returns (q_cat, k_cat, vh) shaped
(B, H, T, 128) each.

Sharding over 8 NeuronCores: 2-way data-parallel over batch (batches {0,1} /
{2,3}) x 4-way tensor-parallel over heads (4 heads per core). Each core runs
one big [8192 x 2048] @ [2048 x 1536] matmul in bf16 (full PE rate, ~3e-3
rel err).

Layout/pipelining choices (from trace analysis):
- W columns are grouped per 512-wide PSUM chunk: chunk0 = [qsem x4 | qgeo x4],
  chunk1 = [ksem x4 | kgeo x4], chunk2 = [v x4]. Each chunk is postprocessed
  independently (uniform strided APs) and DMAd out as soon as it stops, so
  PSUM banks recycle at chunk granularity and the kernel tail is only the
  v-copy + v-DMA of the last m-tile.
- PSUM pool = 8 single-bank tiles; the PE is never more than one bank-free
  wait behind the DVE.
- x is pre-arranged on the host slab-major ([slab, p, k, m]) so each input
  slab is one DMA with 8KB/partition descriptors; slab 0 is split in half so
  the first matmul can start ~2us earlier.
- cos/sin are pre-arranged [p, slot, c] (4KB descriptors), after W on sync.
- W is split across the two HWDGE rings (even k on sync, odd k on scalar)
  and the first slab's matmuls are emitted k-outer so the PE starts as soon
  as slab0a + w0 land (~10us) and works while W streams.
- Staging is t-major ([p, (q|k|v), h, c]) so output DMAs are fully
  contiguous on both sides (1KB descriptors).
"""
import math
import os
import sys

import numpy as np

for _p in ("/opt/trn_rl_repo", os.path.expanduser("~/.axon_site/_ro/trn_rl_repo")):
    if os.path.isdir(_p) and _p not in sys.path:
        sys.path.insert(0, _p)

import ml_dtypes

import concourse.bacc as bacc
import concourse.mybir as mybir
import concourse.tile as tile
from concourse.bass_utils import run_bass_kernel_spmd

# Problem config (hardcoded from the nn.Module init)
D_MODEL = 2048
N_HEADS = 16
SEM_HD = 64
GEO_HD = 64
HEAD_DIM = 128
ROPE_DIM = 64
ROPE_HALF = ROPE_DIM // 2  # 32
ROPE_BASE = 10000.0
B, T = 4, 4096

# Sharding: 2 row groups (2 batches each) x 4 head groups (4 heads each)
N_CORES = 8
RG, HG = 2, 4
ROWS_PER_CORE = (B * T) // RG          # 8192
HEADS_PER_CORE = N_HEADS // HG         # 4
N_CORE = HEADS_PER_CORE * 384          # 1536 packed cols per core
K_TILES = D_MODEL // 128               # 16
M_TILES = ROWS_PER_CORE // 128         # 64
SLAB_MT = 2                            # m_tiles per input DMA slab
SLAB_ROWS = SLAB_MT * 128              # 256
N_SLABS = M_TILES // SLAB_MT           # 32
CHUNK = 512                            # psum bank / matmul moving size
N_CHUNKS = N_CORE // CHUNK             # 3
COS_SLOTS = T // 128                   # 32 distinct cos/sin row-tiles

_f32 = mybir.dt.float32
_bf16 = mybir.dt.bfloat16
_np_bf16 = ml_dtypes.bfloat16


def _build_nc():
    nc = bacc.Bacc("TRN2", target_bir_lowering=False, debug=False, num_devices=1)
    xt_d = nc.dram_tensor(
        "xt", [N_SLABS * 128, K_TILES * SLAB_ROWS], _bf16, kind="ExternalInput"
    )
    w_d = nc.dram_tensor("w", [D_MODEL, N_CORE], _bf16, kind="ExternalInput")
    cos_d = nc.dram_tensor(
        "cos", [128, COS_SLOTS * ROPE_HALF], _f32, kind="ExternalInput"
    )
    sin_d = nc.dram_tensor(
        "sin", [128, COS_SLOTS * ROPE_HALF], _f32, kind="ExternalInput"
    )
    q_d = nc.dram_tensor(
        "q", [ROWS_PER_CORE, HEADS_PER_CORE, HEAD_DIM], _bf16, kind="ExternalOutput"
    )
    k_d = nc.dram_tensor(
        "k", [ROWS_PER_CORE, HEADS_PER_CORE, HEAD_DIM], _bf16, kind="ExternalOutput"
    )
    v_d = nc.dram_tensor(
        "v", [ROWS_PER_CORE, HEADS_PER_CORE, HEAD_DIM], _bf16, kind="ExternalOutput"
    )
    out_ds = (q_d, k_d, v_d)

    with tile.TileContext(nc) as tc:
        with (
            tc.tile_pool(name="wp", bufs=1) as wp,
            tc.tile_pool(name="x0p", bufs=1) as x0p,
            tc.tile_pool(name="xp", bufs=3) as xp,
            tc.tile_pool(name="trig", bufs=1) as trigp,
            tc.tile_pool(name="stg", bufs=3) as stgp,
            tc.tile_pool(name="tmp", bufs=2) as tmpp,
            tc.tile_pool(name="ps", bufs=8, space="PSUM") as ps,
        ):
            xt_sd = xt_d.ap().rearrange("(s p) f -> s p f", p=128)
            slab_tiles = {}

            def load_slab(s):
                if s not in slab_tiles:
                    t = xp.tile([128, K_TILES * SLAB_ROWS], _bf16, tag="xt")
                    # scalar HWDGE ring; one fully contiguous 8KB/partition DMA
                    nc.scalar.dma_start(t[:], xt_sd[s])
                    slab_tiles[s] = t
                return slab_tiles[s]

            # Slab 0 in three pieces (k 0..3 / 4..7 / 8..15) so the ramp's
            # first matmul only waits on 256KB.
            QF = (K_TILES // 4) * SLAB_ROWS
            HALF_F = (K_TILES // 2) * SLAB_ROWS
            xa0 = x0p.tile([128, QF], _bf16, name="xa0", tag="xa0")
            xa1 = x0p.tile([128, QF], _bf16, name="xa1", tag="xa1")
            xb = x0p.tile([128, HALF_F], _bf16, name="xb", tag="xb")
            nc.scalar.dma_start(xa0[:], xt_sd[0][:, 0:QF])

            w_kd = w_d.ap().rearrange("(k p) n -> k p n", p=128)
            w_tiles = [
                wp.tile([128, N_CORE], _bf16, name=f"w{k}", tag=f"w{k}")
                for k in range(K_TILES)
            ]
            cos_sb = trigp.tile([128, COS_SLOTS * ROPE_HALF], _f32, tag="cos")
            sin_sb = trigp.tile([128, COS_SLOTS * ROPE_HALF], _f32, tag="sin")

            # sync ring: even W tiles (w0's first chunk split out so the
            # first matmul can start sooner), then cos/sin (needed ~30us
            # in), then (during the loop) all output DMAs.
            nc.sync.dma_start(w_tiles[0][:, 0:CHUNK], w_kd[0][:, 0:CHUNK])
            nc.sync.dma_start(w_tiles[0][:, CHUNK:], w_kd[0][:, CHUNK:])
            for k in range(2, K_TILES, 2):
                nc.sync.dma_start(w_tiles[k][:], w_kd[k])
            nc.sync.dma_start(cos_sb[:], cos_d.ap())
            nc.sync.dma_start(sin_sb[:], sin_d.ap())
            # scalar ring: xa0, w1, xa1, w3, xb, remaining odd W, slabs 1, 2
            nc.scalar.dma_start(w_tiles[1][:], w_kd[1])
            nc.scalar.dma_start(xa1[:], xt_sd[0][:, QF:HALF_F])
            nc.scalar.dma_start(w_tiles[3][:], w_kd[3])
            nc.scalar.dma_start(xb[:], xt_sd[0][:, HALF_F:])
            for k in range(5, K_TILES, 2):
                nc.scalar.dma_start(w_tiles[k][:], w_kd[k])
            load_slab(1)
            load_slab(2)

            cos_v = cos_sb[:].rearrange("p (s c) -> p s c", s=COS_SLOTS)
            sin_v = sin_sb[:].rearrange("p (s c) -> p s c", s=COS_SLOTS)

            def make_stg():
                return stgp.tile(
                    [128, 3, HEADS_PER_CORE, HEAD_DIM], _bf16,
                    name="stg", tag="stg",
                )

            def _sem_geo_post(t_idx, mt, pc, stg):
                # chunk layout: [sem h0..h3 (256) | geo h0..h3 (256)]
                sv = stg[:]
                slot = mt % COS_SLOTS
                nc.any.tensor_copy(
                    sv[:, t_idx, :, 0:64],
                    pc[:, 0:256].rearrange("p (h c) -> p h c", h=HEADS_PER_CORE),
                )
                xg = pc[:, 256:512].rearrange(
                    "p (h l c) -> p h l c", h=HEADS_PER_CORE, l=2
                )
                x1 = xg[:, :, 0, :]
                x2 = xg[:, :, 1, :]
                cos_bc = (
                    cos_v[:, slot, :]
                    .unsqueeze(1)
                    .broadcast_to([128, HEADS_PER_CORE, ROPE_HALF])
                )
                sin_bc = (
                    sin_v[:, slot, :]
                    .unsqueeze(1)
                    .broadcast_to([128, HEADS_PER_CORE, ROPE_HALF])
                )
                shp = [128, HEADS_PER_CORE, ROPE_HALF]
                t1 = tmpp.tile(shp, _f32, name="t1", tag="t1")
                t2 = tmpp.tile(shp, _f32, name="t2", tag="t2")
                t3 = tmpp.tile(shp, _f32, name="t3", tag="t3")
                t4 = tmpp.tile(shp, _f32, name="t4", tag="t4")
                nc.vector.tensor_mul(t1[:], x1, cos_bc)
                nc.vector.tensor_mul(t2[:], x2, sin_bc)
                nc.vector.tensor_mul(t3[:], x2, cos_bc)
                nc.vector.tensor_mul(t4[:], x1, sin_bc)
                nc.vector.tensor_sub(sv[:, t_idx, :, 64:96], t1[:], t2[:])
                nc.vector.tensor_add(sv[:, t_idx, :, 96:128], t3[:], t4[:])
                m0 = mt * 128
                nc.sync.dma_start(
                    out_ds[t_idx].ap()[m0:m0 + 128, :, :], sv[:, t_idx, :, :]
                )

            def post_q(mt, pc, stg):
                _sem_geo_post(0, mt, pc, stg)

            def post_k(mt, pc, stg):
                _sem_geo_post(1, mt, pc, stg)

            def post_v(mt, pc, stg):
                sv = stg[:]
                nc.any.tensor_copy(
                    sv[:, 2, :, :],
                    pc[:, :].rearrange("p (h c) -> p h c", h=HEADS_PER_CORE),
                )
                m0 = mt * 128
                nc.sync.dma_start(v_d.ap()[m0:m0 + 128, :, :], sv[:, 2, :, :])

            POSTS = (post_q, post_k, post_v)

            # ---- Ramp: slab 0, k-outer so the PE works while W streams ----
            rpts = [
                [
                    ps.tile([128, CHUNK], _f32, name=f"ps{i}{c}", tag="psc")
                    for c in range(N_CHUNKS)
                ]
                for i in range(SLAB_MT)
            ]
            # k emission order = expected W arrival order (even queue leads
            # while the scalar queue moves slab0's pieces).
            ramp_k = [0, 2, 1, 3] + list(range(4, K_TILES))
            for j, k in enumerate(ramp_k):
                if k < 4:
                    xv = xa0[:].rearrange("p (k m) -> p k m", k=K_TILES // 4)
                    xop = xv[:, k, :]
                elif k < 8:
                    xv = xa1[:].rearrange("p (k m) -> p k m", k=K_TILES // 4)
                    xop = xv[:, k - 4, :]
                else:
                    xv = xb[:].rearrange("p (k m) -> p k m", k=K_TILES // 2)
                    xop = xv[:, k - 8, :]
                for i in range(SLAB_MT):
                    for c in range(N_CHUNKS):
                        nc.tensor.matmul(
                            rpts[i][c][:, :],
                            xop[:, i * 128:(i + 1) * 128],
                            w_tiles[k][:, c * CHUNK:(c + 1) * CHUNK],
                            start=(j == 0),
                            stop=(j == K_TILES - 1),
                        )
            for i in range(SLAB_MT):
                stg = make_stg()
                for c in range(N_CHUNKS):
                    POSTS[c](i, rpts[i][c], stg)

            # ---- Steady state ----
            for s in range(1, N_SLABS):
                xt_sb = load_slab(s)
                xt_v = xt_sb[:].rearrange("p (k m) -> p k m", k=K_TILES)
                for i in range(SLAB_MT):
                    mt = s * SLAB_MT + i
                    stg = make_stg()
                    for c in range(N_CHUNKS):
                        pc = ps.tile([128, CHUNK], _f32, name="pc", tag="psc")
                        # k-inner: 16 consecutive accumulating MMs per bank
                        for k in range(K_TILES):
                            nc.tensor.matmul(
                                pc[:, :],
                                xt_v[:, k, i * 128:(i + 1) * 128],
                                w_tiles[k][:, c * CHUNK:(c + 1) * CHUNK],
                                start=(k == 0),
                                stop=(k == K_TILES - 1),
                            )
                        POSTS[c](mt, pc, stg)

    nc.compile()
    return nc


_NC_CACHE = None
LAST_RESULTS = None


def _get_nc():
    global _NC_CACHE
    if _NC_CACHE is None:
        _NC_CACHE = _build_nc()
    return _NC_CACHE


def _host_tables(pos_offset):
    """cos/sin tables computed exactly as the reference does (f32 jax ops),
    pre-arranged to [p, slot, c] so the DMA descriptors are 4KB."""
    import jax
    import jax.numpy as jnp

    with jax.default_device(jax.devices("cpu")[0]):
        inv_freq = ROPE_BASE ** (
            -jnp.arange(0, ROPE_HALF, dtype=jnp.float32) * (2.0 / ROPE_DIM)
        )
        pos = jnp.arange(T, dtype=jnp.float32) + jnp.float32(pos_offset)
        ang = pos[:, None] * inv_freq[None, :]
        cos = np.asarray(jnp.cos(ang), dtype=np.float32)
        sin = np.asarray(jnp.sin(ang), dtype=np.float32)

    def _arr(a):
        # (T, c) -> (p, slot*c) with T = slot*128 + p
        return np.ascontiguousarray(
            a.reshape(COS_SLOTS, 128, ROPE_HALF).transpose(1, 0, 2)
        ).reshape(128, COS_SLOTS * ROPE_HALF)

    return _arr(cos), _arr(sin)


def _gate(gate_logit):
    import jax

    with jax.default_device(jax.devices("cpu")[0]):
        import jax.numpy as jnp

        g = np.asarray(
            jax.nn.sigmoid(jnp.asarray(gate_logit, dtype=jnp.float32)),
            dtype=np.float32,
        )
    return g


def kernel(x, wq_sem, wk_sem, wq_geo, wk_geo, wv, gate_logit, pos_offset):
    x = np.asarray(x, dtype=np.float32)
    wq_sem = np.asarray(wq_sem, dtype=np.float32)
    wk_sem = np.asarray(wk_sem, dtype=np.float32)
    wq_geo = np.asarray(wq_geo, dtype=np.float32)
    wk_geo = np.asarray(wk_geo, dtype=np.float32)
    wv = np.asarray(wv, dtype=np.float32)
    pos_off = int(np.asarray(pos_offset))

    g = _gate(gate_logit)  # (16,)
    sem_scale = np.float32(1.0 / math.sqrt(float(SEM_HD)))
    geo_scale = np.float32(1.0 / math.sqrt(float(GEO_HD)))
    q_sem_col = (np.float32(2.0) * g * sem_scale).astype(np.float32)   # per head
    q_geo_col = ((np.float32(2.0) - np.float32(2.0) * g) * geo_scale).astype(
        np.float32
    )

    # Per-core weight slabs, grouped per 512-chunk:
    # [qsem h0..h3 | qgeo h0..h3] [ksem | kgeo] [v h0..h3]
    w_cores = []
    for hg in range(HG):
        heads = [hg * HEADS_PER_CORE + hl for hl in range(HEADS_PER_CORE)]
        cols = []
        cols += [wq_sem[:, h * 64:(h + 1) * 64] * q_sem_col[h] for h in heads]
        cols += [wq_geo[:, h * 64:(h + 1) * 64] * q_geo_col[h] for h in heads]
        cols += [wk_sem[:, h * 64:(h + 1) * 64] for h in heads]
        cols += [wk_geo[:, h * 64:(h + 1) * 64] for h in heads]
        cols += [wv[:, h * 128:(h + 1) * 128] for h in heads]
        w_cores.append(
            np.ascontiguousarray(np.concatenate(cols, axis=1).astype(_np_bf16))
        )

    # x in bf16, slab-major per core: xt3[s, p, k, ml] = x_rows[s*256+ml, k*128+p]
    xr = x.reshape(B * T, D_MODEL).astype(_np_bf16)
    xt_rg = []
    for rg in range(RG):
        rows = xr[rg * ROWS_PER_CORE:(rg + 1) * ROWS_PER_CORE]
        t = rows.reshape(N_SLABS, SLAB_ROWS, K_TILES, 128)
        xt_rg.append(
            np.ascontiguousarray(t.transpose(0, 3, 2, 1)).reshape(
                N_SLABS * 128, K_TILES * SLAB_ROWS
            )
        )

    cos, sin = _host_tables(pos_off)

    in_maps = []
    for core in range(N_CORES):
        rg, hg = core // HG, core % HG
        in_maps.append(
            {"xt": xt_rg[rg], "w": w_cores[hg], "cos": cos, "sin": sin}
        )

    nc = _get_nc()
    res = run_bass_kernel_spmd(nc, in_maps, list(range(N_CORES)))
    global LAST_RESULTS
    LAST_RESULTS = res

    q_cat = np.empty((B, N_HEADS, T, HEAD_DIM), np.float32)
    k_cat = np.empty((B, N_HEADS, T, HEAD_DIM), np.float32)
    vh = np.empty((B, N_HEADS, T, HEAD_DIM), np.float32)
    for core in range(N_CORES):
        rg, hg = core // HG, core % HG
        r = res.results[core]
        for name, dst in (("q", q_cat), ("k", k_cat), ("v", vh)):
            # (8192, 4, 128) bf16 -> (b_local, heads, T, 128) f32
            a = np.asarray(r[name]).astype(np.float32)
            a = a.reshape(2, T, HEADS_PER_CORE, HEAD_DIM).transpose(0, 2, 1, 3)
            dst[
                rg * 2:(rg + 1) * 2,
                hg * HEADS_PER_CORE:(hg + 1) * HEADS_PER_CORE,
            ] = a
    return q_cat, k_cat, vh


# revision 24
# speedup vs baseline: 1.1745x; 1.0557x over previous
"""Trainium2 Bass kernel for nn_DecoupledAttentionWeight.

Computes the five projections q_sem/k_sem/q_geo/k_geo/v of x, applies RoPE to
the geo paths, the per-head sigmoid gate + per-path scaling to q (folded into
the projection weights host-side), and returns (q_cat, k_cat, vh) shaped
(B, H, T, 128) each.

Sharding over 8 NeuronCores: 2-way data-parallel over batch (batches {0,1} /
{2,3}) x 4-way tensor-parallel over heads (4 heads per core). Each core runs
one big [8192 x 2048] @ [2048 x 1536] matmul in bf16 (full PE rate, ~3e-3
rel err).

Layout/pipelining choices (from trace analysis):
- W columns are grouped per 512-wide PSUM chunk: chunk0 = [qsem x4 | qgeo x4],
  chunk1 = [ksem x4 | kgeo x4], chunk2 = [v x4]. Each chunk is postprocessed
  independently (uniform strided APs) and DMAd out as soon as it stops, so
  PSUM banks recycle at chunk granularity and the kernel tail is only the
  v-copy + v-DMA of the last m-tile.
- PSUM pool = 8 single-bank tiles; the PE is never more than one bank-free
  wait behind the DVE.
- x is pre-arranged on the host slab-major ([slab, p, k, m]) so each input
  slab is one DMA with 8KB/partition descriptors; slab 0 is split in half so
  the first matmul can start ~2us earlier.
- cos/sin are pre-arranged [p, slot, c] (4KB descriptors), after W on sync.
- W is split across the two HWDGE rings (even k on sync, odd k on scalar)
  and the first slab's matmuls are emitted k-outer so the PE starts as soon
  as slab0a + w0 land (~10us) and works while W streams.
- Staging is t-major ([p, (q|k|v), h, c]) so output DMAs are fully
  contiguous on both sides (1KB descriptors).
"""
import math
import os
import sys

import numpy as np

for _p in ("/opt/trn_rl_repo", os.path.expanduser("~/.axon_site/_ro/trn_rl_repo")):
    if os.path.isdir(_p) and _p not in sys.path:
        sys.path.insert(0, _p)

import ml_dtypes

import concourse.bacc as bacc
import concourse.mybir as mybir
import concourse.tile as tile
from concourse.bass_utils import run_bass_kernel_spmd

# Problem config (hardcoded from the nn.Module init)
D_MODEL = 2048
N_HEADS = 16
SEM_HD = 64
GEO_HD = 64
HEAD_DIM = 128
ROPE_DIM = 64
ROPE_HALF = ROPE_DIM // 2  # 32
ROPE_BASE = 10000.0
B, T = 4, 4096

# Sharding: 2 row groups (2 batches each) x 4 head groups (4 heads each)
N_CORES = 8
RG, HG = 2, 4
ROWS_PER_CORE = (B * T) // RG          # 8192
HEADS_PER_CORE = N_HEADS // HG         # 4
N_CORE = HEADS_PER_CORE * 384          # 1536 packed cols per core
K_TILES = D_MODEL // 128               # 16
M_TILES = ROWS_PER_CORE // 128         # 64
SLAB_MT = 2                            # m_tiles per input DMA slab
SLAB_ROWS = SLAB_MT * 128              # 256
N_SLABS = M_TILES // SLAB_MT           # 32
CHUNK = 512                            # psum bank / matmul moving size
N_CHUNKS = N_CORE // CHUNK             # 3
COS_SLOTS = T // 128                   # 32 distinct cos/sin row-tiles

_f32 = mybir.dt.float32
_bf16 = mybir.dt.bfloat16
_fp8 = mybir.dt.float8e4
_np_bf16 = ml_dtypes.bfloat16
_np_fp8 = ml_dtypes.float8_e4m3

# k-tiles 0..13 run in bf16; k-tiles 14+15 run as ONE fp8 DoubleRow matmul
# (K=256 per instruction at ~half the cycles). Total rel err ~1.5e-2 vs the
# 2e-2 gate (fp8 quant error on 2/16 of the contraction).
BF16_KT = K_TILES - 2                  # 14
FP8_KO = 2


def _build_nc():
    nc = bacc.Bacc("TRN2", target_bir_lowering=False, debug=False, num_devices=1)
    xt_d = nc.dram_tensor(
        "xt", [N_SLABS * 128, K_TILES * SLAB_ROWS], _bf16, kind="ExternalInput"
    )
    w_d = nc.dram_tensor("w", [D_MODEL, N_CORE], _bf16, kind="ExternalInput")
    xf8_d = nc.dram_tensor(
        "xf8", [N_SLABS * 128, FP8_KO * SLAB_ROWS], _fp8, kind="ExternalInput"
    )
    wf8_d = nc.dram_tensor(
        "wf8", [128, FP8_KO * N_CORE], _fp8, kind="ExternalInput"
    )
    # per-head q gate/path scales, applied in post_q (NOT folded into W, so
    # the fp8 weight plane stays in e4m3's normal range): [p, 0:4]=sem, [p,
    # 4:8]=geo, replicated across partitions
    qsc_d = nc.dram_tensor("qsc", [128, 2 * HEADS_PER_CORE], _f32,
                           kind="ExternalInput")
    cos_d = nc.dram_tensor(
        "cos", [128, COS_SLOTS * ROPE_HALF], _f32, kind="ExternalInput"
    )
    sin_d = nc.dram_tensor(
        "sin", [128, COS_SLOTS * ROPE_HALF], _f32, kind="ExternalInput"
    )
    q_d = nc.dram_tensor(
        "q", [ROWS_PER_CORE, HEADS_PER_CORE, HEAD_DIM], _bf16, kind="ExternalOutput"
    )
    k_d = nc.dram_tensor(
        "k", [ROWS_PER_CORE, HEADS_PER_CORE, HEAD_DIM], _bf16, kind="ExternalOutput"
    )
    v_d = nc.dram_tensor(
        "v", [ROWS_PER_CORE, HEADS_PER_CORE, HEAD_DIM], _bf16, kind="ExternalOutput"
    )
    out_ds = (q_d, k_d, v_d)

    with tile.TileContext(nc) as tc:
        with (
            tc.tile_pool(name="wp", bufs=1) as wp,
            tc.tile_pool(name="x0p", bufs=1) as x0p,
            tc.tile_pool(name="xp", bufs=3) as xp,
            tc.tile_pool(name="xf8p", bufs=3) as xf8p,
            tc.tile_pool(name="trig", bufs=1) as trigp,
            tc.tile_pool(name="stg", bufs=3) as stgp,
            tc.tile_pool(name="tmp", bufs=2) as tmpp,
            tc.tile_pool(name="ps", bufs=8, space="PSUM") as ps,
        ):
            xt_sd = xt_d.ap().rearrange("(s p) f -> s p f", p=128)
            xf8_sd = xf8_d.ap().rearrange("(s p) f -> s p f", p=128)
            slab_tiles = {}

            def load_slab(s):
                # steady-state slabs only carry the 14 bf16 k-tiles plus the
                # small fp8 plane for k-tiles 14+15
                if s not in slab_tiles:
                    t = xp.tile([128, BF16_KT * SLAB_ROWS], _bf16, tag="xt")
                    # scalar HWDGE ring; one fully contiguous 7KB/partition DMA
                    nc.scalar.dma_start(t[:], xt_sd[s][:, 0:BF16_KT * SLAB_ROWS])
                    t8 = xf8p.tile([128, FP8_KO * SLAB_ROWS], _fp8, tag="x8")
                    nc.scalar.dma_start(t8[:], xf8_sd[s])
                    slab_tiles[s] = (t, t8)
                return slab_tiles[s]

            # Slab 0 in three pieces (k 0..3 / 4..7 / 8..15) so the ramp's
            # first matmul only waits on 256KB.
            QF = (K_TILES // 4) * SLAB_ROWS
            HALF_F = (K_TILES // 2) * SLAB_ROWS
            xa0 = x0p.tile([128, QF], _bf16, name="xa0", tag="xa0")
            xa1 = x0p.tile([128, QF], _bf16, name="xa1", tag="xa1")
            xb = x0p.tile([128, HALF_F], _bf16, name="xb", tag="xb")
            nc.scalar.dma_start(xa0[:], xt_sd[0][:, 0:QF])

            w_kd = w_d.ap().rearrange("(k p) n -> k p n", p=128)
            w_tiles = [
                wp.tile([128, N_CORE], _bf16, name=f"w{k}", tag=f"w{k}")
                for k in range(K_TILES)
            ]
            cos_sb = trigp.tile([128, COS_SLOTS * ROPE_HALF], _f32, tag="cos")
            sin_sb = trigp.tile([128, COS_SLOTS * ROPE_HALF], _f32, tag="sin")

            # sync ring: even W tiles (w0's first chunk split out so the
            # first matmul can start sooner), then cos/sin (needed ~30us
            # in), then (during the loop) all output DMAs.
            nc.sync.dma_start(w_tiles[0][:, 0:CHUNK], w_kd[0][:, 0:CHUNK])
            nc.sync.dma_start(w_tiles[0][:, CHUNK:], w_kd[0][:, CHUNK:])
            for k in range(2, K_TILES, 2):
                nc.sync.dma_start(w_tiles[k][:], w_kd[k])
            nc.sync.dma_start(cos_sb[:], cos_d.ap())
            nc.sync.dma_start(sin_sb[:], sin_d.ap())
            qsc_sb = trigp.tile([128, 2 * HEADS_PER_CORE], _f32, tag="qsc")
            nc.sync.dma_start(qsc_sb[:], qsc_d.ap())
            wf8_t = wp.tile([128, FP8_KO * N_CORE], _fp8, name="wf8", tag="wf8")
            nc.sync.dma_start(wf8_t[:], wf8_d.ap())
            # scalar ring: xa0, w1, xa1, w3, xb, remaining odd W, slabs 1, 2
            nc.scalar.dma_start(w_tiles[1][:], w_kd[1])
            nc.scalar.dma_start(xa1[:], xt_sd[0][:, QF:HALF_F])
            nc.scalar.dma_start(w_tiles[3][:], w_kd[3])
            nc.scalar.dma_start(xb[:], xt_sd[0][:, HALF_F:])
            for k in range(5, K_TILES, 2):
                nc.scalar.dma_start(w_tiles[k][:], w_kd[k])
            load_slab(1)
            load_slab(2)

            cos_v = cos_sb[:].rearrange("p (s c) -> p s c", s=COS_SLOTS)
            sin_v = sin_sb[:].rearrange("p (s c) -> p s c", s=COS_SLOTS)

            def make_stg():
                return stgp.tile(
                    [128, 3, HEADS_PER_CORE, HEAD_DIM], _bf16,
                    name="stg", tag="stg",
                )

            def _sem_geo_post(t_idx, mt, pc, stg, scaled):
                # chunk layout: [sem h0..h3 (256) | geo h0..h3 (256)]
                sv = stg[:]
                slot = mt % COS_SLOTS
                sem_src = pc[:, 0:256].rearrange(
                    "p (h c) -> p h c", h=HEADS_PER_CORE
                )
                if scaled:
                    sem_bc = (
                        qsc_sb[:, 0:HEADS_PER_CORE]
                        .unsqueeze(2)
                        .broadcast_to([128, HEADS_PER_CORE, 64])
                    )
                    nc.vector.tensor_mul(sv[:, t_idx, :, 0:64], sem_src, sem_bc)
                else:
                    nc.any.tensor_copy(sv[:, t_idx, :, 0:64], sem_src)
                xg = pc[:, 256:512].rearrange(
                    "p (h l c) -> p h l c", h=HEADS_PER_CORE, l=2
                )
                x1 = xg[:, :, 0, :]
                x2 = xg[:, :, 1, :]
                cos_bc = (
                    cos_v[:, slot, :]
                    .unsqueeze(1)
                    .broadcast_to([128, HEADS_PER_CORE, ROPE_HALF])
                )
                sin_bc = (
                    sin_v[:, slot, :]
                    .unsqueeze(1)
                    .broadcast_to([128, HEADS_PER_CORE, ROPE_HALF])
                )
                shp = [128, HEADS_PER_CORE, ROPE_HALF]
                t1 = tmpp.tile(shp, _f32, name="t1", tag="t1")
                t2 = tmpp.tile(shp, _f32, name="t2", tag="t2")
                t3 = tmpp.tile(shp, _f32, name="t3", tag="t3")
                t4 = tmpp.tile(shp, _f32, name="t4", tag="t4")
                nc.vector.tensor_mul(t1[:], x1, cos_bc)
                nc.vector.tensor_mul(t2[:], x2, sin_bc)
                nc.vector.tensor_mul(t3[:], x2, cos_bc)
                nc.vector.tensor_mul(t4[:], x1, sin_bc)
                if scaled:
                    geo_bc = (
                        qsc_sb[:, HEADS_PER_CORE:2 * HEADS_PER_CORE]
                        .unsqueeze(2)
                        .broadcast_to([128, HEADS_PER_CORE, ROPE_HALF])
                    )
                    r1 = tmpp.tile(shp, _f32, name="r1", tag="r1")
                    r2 = tmpp.tile(shp, _f32, name="r2", tag="r2")
                    nc.vector.tensor_sub(r1[:], t1[:], t2[:])
                    nc.vector.tensor_add(r2[:], t3[:], t4[:])
                    nc.vector.tensor_mul(sv[:, t_idx, :, 64:96], r1[:], geo_bc)
                    nc.vector.tensor_mul(sv[:, t_idx, :, 96:128], r2[:], geo_bc)
                else:
                    nc.vector.tensor_sub(sv[:, t_idx, :, 64:96], t1[:], t2[:])
                    nc.vector.tensor_add(sv[:, t_idx, :, 96:128], t3[:], t4[:])
                m0 = mt * 128
                nc.sync.dma_start(
                    out_ds[t_idx].ap()[m0:m0 + 128, :, :], sv[:, t_idx, :, :]
                )

            def post_q(mt, pc, stg):
                _sem_geo_post(0, mt, pc, stg, scaled=True)

            def post_k(mt, pc, stg):
                _sem_geo_post(1, mt, pc, stg, scaled=False)

            def post_v(mt, pc, stg):
                sv = stg[:]
                nc.any.tensor_copy(
                    sv[:, 2, :, :],
                    pc[:, :].rearrange("p (h c) -> p h c", h=HEADS_PER_CORE),
                )
                m0 = mt * 128
                nc.sync.dma_start(v_d.ap()[m0:m0 + 128, :, :], sv[:, 2, :, :])

            POSTS = (post_q, post_k, post_v)

            # ---- Ramp: slab 0, k-outer so the PE works while W streams ----
            rpts = [
                [
                    ps.tile([128, CHUNK], _f32, name=f"ps{i}{c}", tag="psc")
                    for c in range(N_CHUNKS)
                ]
                for i in range(SLAB_MT)
            ]
            # k emission order = expected W arrival order (even queue leads
            # while the scalar queue moves slab0's pieces).
            ramp_k = [0, 2, 1, 3] + list(range(4, K_TILES))
            for j, k in enumerate(ramp_k):
                if k < 4:
                    xv = xa0[:].rearrange("p (k m) -> p k m", k=K_TILES // 4)
                    xop = xv[:, k, :]
                elif k < 8:
                    xv = xa1[:].rearrange("p (k m) -> p k m", k=K_TILES // 4)
                    xop = xv[:, k - 4, :]
                else:
                    xv = xb[:].rearrange("p (k m) -> p k m", k=K_TILES // 2)
                    xop = xv[:, k - 8, :]
                for i in range(SLAB_MT):
                    for c in range(N_CHUNKS):
                        nc.tensor.matmul(
                            rpts[i][c][:, :],
                            xop[:, i * 128:(i + 1) * 128],
                            w_tiles[k][:, c * CHUNK:(c + 1) * CHUNK],
                            start=(j == 0),
                            stop=(j == K_TILES - 1),
                        )
            for i in range(SLAB_MT):
                stg = make_stg()
                for c in range(N_CHUNKS):
                    POSTS[c](i, rpts[i][c], stg)

            # ---- Steady state ----
            wf8_v = wf8_t[:].rearrange("p (o n) -> p o n", o=FP8_KO)
            for s in range(1, N_SLABS):
                xt_sb, xf8_sb = load_slab(s)
                xt_v = xt_sb[:].rearrange("p (k m) -> p k m", k=BF16_KT)
                xf8_v = xf8_sb[:].rearrange("p (o m) -> p o m", o=FP8_KO)
                for i in range(SLAB_MT):
                    mt = s * SLAB_MT + i
                    stg = make_stg()
                    for c in range(N_CHUNKS):
                        pc = ps.tile([128, CHUNK], _f32, name="pc", tag="psc")
                        # 14 bf16 accumulating MMs, then one fp8 DoubleRow MM
                        # covering k-tiles 14+15 (K=256) closes the group.
                        for k in range(BF16_KT):
                            nc.tensor.matmul(
                                pc[:, :],
                                xt_v[:, k, i * 128:(i + 1) * 128],
                                w_tiles[k][:, c * CHUNK:(c + 1) * CHUNK],
                                start=(k == 0),
                                stop=False,
                            )
                        nc.tensor.matmul(
                            pc[:, :],
                            xf8_v[:, :, i * 128:(i + 1) * 128],
                            wf8_v[:, :, c * CHUNK:(c + 1) * CHUNK],
                            start=False,
                            stop=True,
                            perf_mode=mybir.MatmulPerfMode.DoubleRow,
                        )
                        POSTS[c](mt, pc, stg)

    nc.compile()
    return nc


_NC_CACHE = None
LAST_RESULTS = None


def _get_nc():
    global _NC_CACHE
    if _NC_CACHE is None:
        _NC_CACHE = _build_nc()
    return _NC_CACHE


def _host_tables(pos_offset):
    """cos/sin tables computed exactly as the reference does (f32 jax ops),
    pre-arranged to [p, slot, c] so the DMA descriptors are 4KB."""
    import jax
    import jax.numpy as jnp

    with jax.default_device(jax.devices("cpu")[0]):
        inv_freq = ROPE_BASE ** (
            -jnp.arange(0, ROPE_HALF, dtype=jnp.float32) * (2.0 / ROPE_DIM)
        )
        pos = jnp.arange(T, dtype=jnp.float32) + jnp.float32(pos_offset)
        ang = pos[:, None] * inv_freq[None, :]
        cos = np.asarray(jnp.cos(ang), dtype=np.float32)
        sin = np.asarray(jnp.sin(ang), dtype=np.float32)

    def _arr(a):
        # (T, c) -> (p, slot*c) with T = slot*128 + p
        return np.ascontiguousarray(
            a.reshape(COS_SLOTS, 128, ROPE_HALF).transpose(1, 0, 2)
        ).reshape(128, COS_SLOTS * ROPE_HALF)

    return _arr(cos), _arr(sin)


def _gate(gate_logit):
    import jax

    with jax.default_device(jax.devices("cpu")[0]):
        import jax.numpy as jnp

        g = np.asarray(
            jax.nn.sigmoid(jnp.asarray(gate_logit, dtype=jnp.float32)),
            dtype=np.float32,
        )
    return g


def kernel(x, wq_sem, wk_sem, wq_geo, wk_geo, wv, gate_logit, pos_offset):
    x = np.asarray(x, dtype=np.float32)
    wq_sem = np.asarray(wq_sem, dtype=np.float32)
    wk_sem = np.asarray(wk_sem, dtype=np.float32)
    wq_geo = np.asarray(wq_geo, dtype=np.float32)
    wk_geo = np.asarray(wk_geo, dtype=np.float32)
    wv = np.asarray(wv, dtype=np.float32)
    pos_off = int(np.asarray(pos_offset))

    g = _gate(gate_logit)  # (16,)
    sem_scale = np.float32(1.0 / math.sqrt(float(SEM_HD)))
    geo_scale = np.float32(1.0 / math.sqrt(float(GEO_HD)))
    q_sem_col = (np.float32(2.0) * g * sem_scale).astype(np.float32)   # per head
    q_geo_col = ((np.float32(2.0) - np.float32(2.0) * g) * geo_scale).astype(
        np.float32
    )

    # Per-core weight slabs, grouped per 512-chunk:
    # [qsem h0..h3 | qgeo h0..h3] [ksem | kgeo] [v h0..h3]
    # q gate/path scales are NOT folded in (post_q applies them), so the
    # fp8 plane keeps full e4m3 precision. fp8 split: w*8, x/8 to center
    # both operands in e4m3's normal range.
    FP8_WS = np.float32(8.0)
    w_cores = []
    qsc_cores = []
    for hg in range(HG):
        heads = [hg * HEADS_PER_CORE + hl for hl in range(HEADS_PER_CORE)]
        cols = []
        cols += [wq_sem[:, h * 64:(h + 1) * 64] for h in heads]
        cols += [wq_geo[:, h * 64:(h + 1) * 64] for h in heads]
        cols += [wk_sem[:, h * 64:(h + 1) * 64] for h in heads]
        cols += [wk_geo[:, h * 64:(h + 1) * 64] for h in heads]
        cols += [wv[:, h * 128:(h + 1) * 128] for h in heads]
        wc = np.concatenate(cols, axis=1)
        # fp8 plane for k-tiles 14+15: [p, ko, n] packed for DoubleRow
        wf = (wc[BF16_KT * 128:, :] * FP8_WS).reshape(FP8_KO, 128, N_CORE)
        w_cores.append(
            (
                np.ascontiguousarray(wc.astype(_np_bf16)),
                np.ascontiguousarray(wf.transpose(1, 0, 2)).reshape(
                    128, FP8_KO * N_CORE
                ).astype(_np_fp8),
            )
        )
        qsc = np.concatenate([q_sem_col[heads], q_geo_col[heads]])
        qsc_cores.append(
            np.ascontiguousarray(np.broadcast_to(qsc[None, :], (128, 8)))
        )

    # x in bf16, slab-major per core: xt3[s, p, k, ml] = x_rows[s*256+ml, k*128+p]
    xr = x.reshape(B * T, D_MODEL).astype(_np_bf16)
    xt_rg = []
    xf8_rg = []
    for rg in range(RG):
        rows = xr[rg * ROWS_PER_CORE:(rg + 1) * ROWS_PER_CORE]
        t = rows.reshape(N_SLABS, SLAB_ROWS, K_TILES, 128)
        xt_rg.append(
            np.ascontiguousarray(t.transpose(0, 3, 2, 1)).reshape(
                N_SLABS * 128, K_TILES * SLAB_ROWS
            )
        )
        # fp8 plane for k-tiles 14+15 (scaled by 1/8): [s, p, ko, ml]
        t8 = rows[:, BF16_KT * 128:].astype(np.float32) * np.float32(0.125)
        t8 = t8.reshape(N_SLABS, SLAB_ROWS, FP8_KO, 128)
        xf8_rg.append(
            np.ascontiguousarray(t8.transpose(0, 3, 2, 1)).reshape(
                N_SLABS * 128, FP8_KO * SLAB_ROWS
            ).astype(_np_fp8)
        )

    cos, sin = _host_tables(pos_off)

    in_maps = []
    for core in range(N_CORES):
        rg, hg = core // HG, core % HG
        in_maps.append(
            {
                "xt": xt_rg[rg],
                "w": w_cores[hg][0],
                "xf8": xf8_rg[rg],
                "wf8": w_cores[hg][1],
                "qsc": qsc_cores[hg],
                "cos": cos,
                "sin": sin,
            }
        )

    nc = _get_nc()
    res = run_bass_kernel_spmd(nc, in_maps, list(range(N_CORES)))
    global LAST_RESULTS
    LAST_RESULTS = res

    q_cat = np.empty((B, N_HEADS, T, HEAD_DIM), np.float32)
    k_cat = np.empty((B, N_HEADS, T, HEAD_DIM), np.float32)
    vh = np.empty((B, N_HEADS, T, HEAD_DIM), np.float32)
    for core in range(N_CORES):
        rg, hg = core // HG, core % HG
        r = res.results[core]
        for name, dst in (("q", q_cat), ("k", k_cat), ("v", vh)):
            # (8192, 4, 128) bf16 -> (b_local, heads, T, 128) f32
            a = np.asarray(r[name]).astype(np.float32)
            a = a.reshape(2, T, HEADS_PER_CORE, HEAD_DIM).transpose(0, 2, 1, 3)
            dst[
                rg * 2:(rg + 1) * 2,
                hg * HEADS_PER_CORE:(hg + 1) * HEADS_PER_CORE,
            ] = a
    return q_cat, k_cat, vh
